# revision 5
# baseline (speedup 1.0000x reference)
"""Trainium2 Bass kernel for batched multi-head attention.

HW exec time: ~263us full-clock (v5 baseline 264us full-clock / 313us
throttled). Verified rel err 0.0037.

Distribution: pure data parallel - batch b -> NeuronCore b (B=8=cores,
zero collectives; TP would add communication without reducing FLOPs).

v6 vs v5 (264us full-clock):
  - ALL projection matmuls (prefix proj_full, kq/v halves, output
    projection) col-tiled into (128x64)-mode pairs, tile_position
    (0,0)/(0,64): tiled-mode rounds chain with ~0ns gaps while full
    128x128-mode matmuls pay ~65ns/MM swap gap (measured 77ns
    throttled); pairs run concurrently (2nd MM of a pair shows ~4ns).
  - per-group PE emission [Z (128x32)] [AV + flush + fillers (128x64)]
    [scores (64x128)]: 3 tiling-mode switches per group (each switch
    drains the PE, ~105-190ns), and scores go LAST so a full group of
    trail/filler work is queued while the PE waits for exp(g-1) to
    free the scores PSUM pool (scores-first head-of-line blocked).
  - Z widened to 32 redundant out-rows (ones [128,32] lhsT, same
    cycles) so zt[128,512] is fully written -> reciprocal reads no
    garbage -> flush broadcast via sel-matrix lhsT (zeros with one
    ones-row) runs in the same (128x64) mode as AV (no mode switch).
  - avr (attention numerator) eviction split into two 512-col halves
    emitted between exp c0/c1 of consecutive groups: a full copy in
    the scalar queue delays exp c1 past the scores-PSUM-free deadline
    (observed ~615ns scores stalls once per row); at row starts AV is
    emitted after the fillers so the av banks are needed ~2.3us in.
  - input DMAs ordered by first use: x+wv whole tiles, then the col-0
    chunks of wk/wq (all K0/Q0 need), masks, wk/wq remainder, wo.
    (Do NOT stripe tiles into small column chunks: <2KB per-partition
    DMA lines collapse DMA bandwidth; cost ~20us when masks lag.)
  - ~4us HAM warmup matmul chain on memset data during the DMA wait
    (PE clock-gate opens 1.2->2.4GHz after ~3.4us of activity).
  - flush halves at r==6/r==10 (q0 filler at r==8) so consecutive
    users of the single work PSUM bank are >=2 groups apart.

PSUM: scores 2x[128,1024] (4 banks), AV [128,1024] (2), Z [128,512]
(1), work [128,512] (1) = 8 banks.

Steady state (full clock): group period ~2.85us; PE ~2.4us/group
(Z 1 round, AV 2, scores 2, fillers ~4.5 at ~218ns/round + ~330ns
switch overhead), scalar 2x1035ns exp (the secondary critical chain),
vector ~1.4-2.4us (mask muls, psum evictions, reciprocals).
GpSimd unusable for evictions: it cannot access PSUM on trn2.
"""

import sys

if "/opt/trn_rl_repo" not in sys.path:
    sys.path.insert(0, "/opt/trn_rl_repo")

import numpy as np
import ml_dtypes

B = 8
L = 1024
D = 1024
H = 16
DH = 64
NT = 8
SCALE = 0.125
EXP_SHIFT = -32.0
TRAIL = 6  # AV/Z trail the scores by this many steps (even: aligns mt pairs)
N_CORES = 8
NSTEP = 2 * NT

_NC_CACHE = None


def _build():
    import concourse.bass as bass  # noqa: F401
    from concourse import bacc, mybir
    import concourse.tile as tile

    f32 = mybir.dt.float32
    f16 = mybir.dt.float16
    bf16 = mybir.dt.bfloat16
    Exp = mybir.ActivationFunctionType.Exp

    nc = bacc.Bacc(None, target_bir_lowering=False)

    xT = nc.declare_dram_parameter("xT", [D, L], f16, isOutput=False)
    wqT = nc.declare_dram_parameter("wqT", [D, D], f16, isOutput=False)
    wkT = nc.declare_dram_parameter("wkT", [D, D], f16, isOutput=False)
    wvT = nc.declare_dram_parameter("wvT", [D, D], bf16, isOutput=False)
    woT = nc.declare_dram_parameter("woT", [D, D], f16, isOutput=False)
    mskT = nc.declare_dram_parameter("maskT", [L, L], bf16, isOutput=False)
    out = nc.declare_dram_parameter("out", [L, D], f16, isOutput=True)

    with tile.TileContext(nc) as tc:
        with (
            tc.tile_pool(name="big", bufs=1) as big,
            tc.tile_pool(name="pb", bufs=TRAIL + 3) as pbp,
            tc.tile_pool(name="sm", bufs=2) as smp,
            tc.tile_pool(name="rb", bufs=2) as rbp,
            tc.tile_pool(name="avr", bufs=2) as avrp,
            tc.tile_pool(name="yb", bufs=2) as ybp,
            tc.tile_pool(name="sc", bufs=2, space="PSUM") as scp,
            tc.tile_pool(name="pav", bufs=1, space="PSUM") as pavp,
            tc.tile_pool(name="pz", bufs=1, space="PSUM") as pzp,
            tc.tile_pool(name="wk", bufs=1, space="PSUM") as wkp,
        ):
            x_t = [big.tile([128, L], f16, tag=f"x{i}", name=f"x{i}") for i in range(NT)]
            wq_t = [big.tile([128, D], f16, tag=f"wq{i}", name=f"wq{i}") for i in range(NT)]
            wk_t = [big.tile([128, D], f16, tag=f"wk{i}", name=f"wk{i}") for i in range(NT)]
            wv_t = [big.tile([128, D], bf16, tag=f"wv{i}", name=f"wv{i}") for i in range(NT)]
            wo_t = [big.tile([128, D], f16, tag=f"wo{i}", name=f"wo{i}") for i in range(NT)]
            mk_t = [big.tile([128, L], bf16, tag=f"mk{i}", name=f"mk{i}") for i in range(NT)]
            qt_t = [big.tile([128, L], f16, tag=f"qt{i}", name=f"qt{i}") for i in range(NT)]
            kt_t = [big.tile([128, L], f16, tag=f"kt{i}", name=f"kt{i}") for i in range(NT)]
            v_t = [big.tile([128, H, DH], bf16, tag=f"v{i}", name=f"v{i}") for i in range(NT)]
            ot_t = [big.tile([128, L], f16, tag=f"ot{i}", name=f"ot{i}") for i in range(NT)]
            ones32 = big.tile([128, 32], bf16, tag="ones32")
            sel = big.tile([128, 4, 64], bf16, tag="sel")
            negC = big.tile([128, 1], f32, tag="negC")

            # input DMAs, ordered by first use and striped so early tiles
            # land first: x+wv quarters (V0 starts ~3us in), then the col-0
            # chunks of wk/wq (all K0/Q0 needs), mask tiles, then the rest.
            # wv whole + the x column chunk V0/V1 contract over: V0 can
            # start at ~5us instead of waiting for all of x (~8.5us)
            for i in range(NT):
                sl = slice(i * 128, (i + 1) * 128)
                nc.sync.dma_start(out=wv_t[i][:, :], in_=wvT[sl, :])
                nc.sync.dma_start(out=x_t[i][:, 0:256], in_=xT[sl, 0:256])
            for i in range(NT):
                sl = slice(i * 128, (i + 1) * 128)
                nc.sync.dma_start(out=wk_t[i][:, 0:128], in_=wkT[sl, 0:128])
                nc.sync.dma_start(out=wq_t[i][:, 0:128], in_=wqT[sl, 0:128])
            for i in range(NT):
                sl = slice(i * 128, (i + 1) * 128)
                nc.sync.dma_start(out=x_t[i][:, 256:1024], in_=xT[sl, 256:1024])
            for i in range(NT):
                sl = slice(i * 128, (i + 1) * 128)
                nc.sync.dma_start(out=mk_t[i][:, :], in_=mskT[sl, :])
            for i in range(NT):
                sl = slice(i * 128, (i + 1) * 128)
                nc.sync.dma_start(out=wk_t[i][:, 128:1024], in_=wkT[sl, 128:1024])
                nc.sync.dma_start(out=wq_t[i][:, 128:1024], in_=wqT[sl, 128:1024])
            for i in range(NT):
                sl = slice(i * 128, (i + 1) * 128)
                nc.sync.dma_start(out=wo_t[i][:, :], in_=woT[sl, :])

            nc.vector.memset(ones32[:, :], 1.0)
            nc.vector.memset(sel[:, :, :], 0.0)
            for q in range(4):
                nc.vector.memset(sel[32 * q : 32 * q + 1, q, :], 1.0)
            nc.vector.memset(negC[:, :], EXP_SHIFT)

            def proj_full(lhs_fn, rhs_fn, evict):
                # col-tiled (128x64 mode): two concurrent 64-col tiles per
                # 512-col step; tiled rounds chain with ~0 gap vs 77ns/MM in
                # full 128x128 mode.
                ps = scp.tile([128, L], f32, tag="sc", name="projps")
                for c in range(2):
                    cs = slice(c * 512, (c + 1) * 512)
                    for dt in range(NT):
                        for h in range(2):
                            nc.tensor.matmul(
                                ps[64 * h : 64 * (h + 1), cs],
                                lhsT=lhs_fn(dt, h),
                                rhs=rhs_fn(dt, cs),
                                start=(dt == 0),
                                stop=(dt == NT - 1),
                                tile_position=(0, 64 * h),
                            )
                evict(ps)

            def v_proj(mt):
                proj_full(
                    lambda dt, h: x_t[dt][:, mt * 128 + 64 * h : mt * 128 + 64 * (h + 1)],
                    lambda dt, cs: wv_t[dt][:, cs],
                    lambda ps: nc.vector.tensor_copy(
                        out=v_t[mt][:, :, :],
                        in_=ps[:, :].rearrange("p (h e) -> p h e", h=H),
                    ),
                )

            def kq_proj(w_tiles, dst, et):
                proj_full(
                    lambda dt, h: w_tiles[dt][:, et * 128 + 64 * h : et * 128 + 64 * (h + 1)],
                    lambda dt, cs: x_t[dt][:, cs],
                    lambda ps: nc.vector.tensor_copy(out=dst[et][:, :], in_=ps[:, :]),
                )

            def kq_proj_half(w_tiles, dst, et, c):
                ps = wkp.tile([128, 512], f32, tag="wk", name="hp")
                cs = slice(c * 512, (c + 1) * 512)
                for dt in range(NT):
                    for h in range(2):
                        nc.tensor.matmul(
                            ps[64 * h : 64 * (h + 1), :],
                            lhsT=w_tiles[dt][
                                :, et * 128 + 64 * h : et * 128 + 64 * (h + 1)
                            ],
                            rhs=x_t[dt][:, cs],
                            start=(dt == 0),
                            stop=(dt == NT - 1),
                            tile_position=(0, 64 * h),
                        )
                nc.vector.tensor_copy(out=dst[et][:, cs], in_=ps[:, :])

            def v_proj_half(mt, c):
                # half of V proj: output heads c*8..c*8+8 of v_t[mt]
                ps = wkp.tile([128, 512], f32, tag="wk", name="vh")
                cs = slice(c * 512, (c + 1) * 512)
                for dt in range(NT):
                    for h in range(2):
                        nc.tensor.matmul(
                            ps[64 * h : 64 * (h + 1), :],
                            lhsT=x_t[dt][
                                :, mt * 128 + 64 * h : mt * 128 + 64 * (h + 1)
                            ],
                            rhs=wv_t[dt][:, cs],
                            start=(dt == 0),
                            stop=(dt == NT - 1),
                            tile_position=(0, 64 * h),
                        )
                nc.vector.tensor_copy(
                    out=v_t[mt][:, c * 8 : (c + 1) * 8, :],
                    in_=ps[:, :].rearrange("p (h e) -> p h e", h=8),
                )

            # ---- prefix: V0,V1,K0,V2,V3,Q0 (V4..V7 run inside row 0)
            v_proj(0)
            v_proj(1)
            kq_proj(wk_t, kt_t, 0)
            v_proj(2)
            v_proj(3)
            kq_proj(wq_t, qt_t, 0)

            # ---- head: 64 groups of 2 steps: (et, mt) with c=0,1
            def flush_half(p, c, rb):
                # broadcast reciprocal rows via sel matrices in the same
                # (128,64) col-tiled mode as AV/fillers (no PE mode switch)
                pet, pavr, przb = p
                cs = slice(c * 512, (c + 1) * 512)
                psr = wkp.tile([128, 512], f32, tag="wk", name="psr")
                for j in range(2):
                    q = 2 * j + c
                    nc.tensor.matmul(
                        psr[64 * j : 64 * j + 64, :],
                        lhsT=sel[:, q, :],
                        rhs=przb[:, :],
                        start=True,
                        stop=True,
                        tile_position=(0, 64 * j),
                    )
                nc.vector.tensor_copy(out=rb[:, cs], in_=psr[:, :])
                if c == 1:
                    nc.vector.tensor_mul(ot_t[pet][:, :], pavr[:, :], rb[:, :])

            pending = None
            flushing = None
            rb_cur = None
            av = None
            zt = None
            avr_jobs = []
            deferred_av = None
            pref = {}
            NSTEPS = NT * NT * 2

            # ---- HAM warmup: ~4us of dummy matmuls on resident data while
            # the input DMAs stream, so the PE clock-gate opens (1.2->2.4GHz)
            # before the first real projection.
            wps = wkp.tile([128, 512], f32, tag="wk", name="warm")
            for wi in range(20):
                nc.tensor.matmul(
                    wps[0:64, 0:256],
                    lhsT=sel[:, 0, :],
                    rhs=sel[:, :, :].rearrange("p a b -> p (a b)"),
                    start=(wi == 0),
                    stop=(wi == 19),
                    tile_position=(0, 0),
                )
            # per-group emission: [Z (128x32)] [AV+flush+fillers (128x64)]
            # [scores (64x128)] — 3 mode switches/group, and scores go last
            # so the PE has a full group of work queued while waiting for
            # exp(g-1) to free the scores PSUM pool.
            for s2 in range(0, NSTEPS + TRAIL, 2):
                as2 = s2 - TRAIL
                if as2 >= 0:
                    aet, ar = divmod(as2, NSTEP)
                    amt = ar // 2
                    if ar == 0:
                        av = pavp.tile([128, L], f32, tag="av", name="av")
                        zt = pzp.tile([128, 512], f32, tag="z", name="z")
                    p2c = [pref.pop((aet, ar)), pref.pop((aet, ar + 1))]
                    for j in range(2):
                        for cc in range(2):
                            q = 2 * j + cc
                            nc.tensor.matmul(
                                zt[32 * q : 32 * q + 32, :],
                                lhsT=ones32[:, :],
                                rhs=p2c[cc][:, j, :],
                                start=(amt == 0),
                                stop=(amt == NT - 1),
                                tile_position=(0, 32 * q),
                            )

                    def emit_av(amt=amt, aet=aet, p2c=p2c, av=av):
                        for c in range(2):
                            acs = slice(c * 512, (c + 1) * 512)
                            for j in range(2):
                                nc.tensor.matmul(
                                    av[j * 64 : (j + 1) * 64, acs],
                                    lhsT=v_t[amt][:, 2 * aet + j, :],
                                    rhs=p2c[c][:, j, :],
                                    start=(amt == 0),
                                    stop=(amt == NT - 1),
                                    tile_position=(0, 64 * j),
                                )

                    # at row starts the av banks are freed by the avr copy
                    # only ~1.5us into the group: emit AV after the fillers
                    # so the PE doesn't stall on the bank WAR
                    if ar == 0 and s2 < NSTEPS:
                        deferred_av = emit_av
                    else:
                        emit_av()
                    if ar == NSTEP - 2:
                        # avr eviction: two 512-col halves slotted between
                        # the exps of this and the next group (each half
                        # fits the exp-chain slack; a full copy would delay
                        # exp c1 enough to stall the next group's scores)
                        avr = avrp.tile([128, L], bf16, tag="avr", name="avr")
                        if s2 >= NSTEPS - 2:
                            nc.scalar.copy(out=avr[:, :], in_=av[:, :])
                        else:
                            for hc in range(2):
                                hs = slice(hc * 512, (hc + 1) * 512)
                                avr_jobs.append(
                                    lambda hs=hs, avr=avr, av=av: nc.scalar.copy(
                                        out=avr[:, hs], in_=av[:, hs]
                                    )
                                )
                        rz = smp.tile([128, 512], f32, tag="rz", name="rz")
                        nc.vector.reciprocal_approx_fast(out=rz[:, :], in_=zt[:, :])
                        rzb = smp.tile([128, 512], bf16, tag="rzb", name="rzb")
                        nc.vector.tensor_copy(out=rzb[:, :], in_=rz[:, :])
                        pending = (aet, avr, rzb)
                if s2 < NSTEPS:
                    et, r = divmod(s2, NSTEP)
                    mt = r // 2
                    if r == 6 and pending is not None:
                        flushing = pending
                        pending = None
                        rb_cur = rbp.tile([128, L], bf16, name="rb")
                        flush_half(flushing, 0, rb_cur)
                    if r == 10 and flushing is not None:
                        flush_half(flushing, 1, rb_cur)
                        flushing = None
                    # fillers: row 0 hosts V4..V7 c0-halves (heads 0-7,
                    # all rows 0-3 need only these); c1-halves spread over
                    # rows 0-3 (first needed by row 4)
                    if et == 0 and r <= 6:
                        v_proj_half(4 + r // 2, 0)
                    if et == 0 and r == 8:
                        v_proj_half(4, 1)
                    if 1 <= et <= 3 and r == 0:
                        v_proj_half(4 + et, 1)
                    if r == 4 and et >= 1:
                        # current row's K c1 half (needed from r=8)
                        kq_proj_half(wk_t, kt_t, et, 1)
                    if et + 1 < NT:
                        if r == (8 if et >= 1 else 10):
                            kq_proj_half(wq_t, qt_t, et + 1, 0)
                        elif r == 12:
                            kq_proj_half(wq_t, qt_t, et + 1, 1)
                        elif r == 14:
                            kq_proj_half(wk_t, kt_t, et + 1, 0)
                    if deferred_av is not None:
                        deferred_av()
                        deferred_av = None
                    # scores last (64x128 row-tiled mode)
                    pscs = []
                    for c in range(2):
                        psc = scp.tile([128, L], f32, tag="sc", name="sps")
                        pscs.append(psc)
                    for c in range(2):
                        cs = slice(c * 512, (c + 1) * 512)
                        for j in range(2):
                            jp = slice(j * 64, (j + 1) * 64)
                            nc.tensor.matmul(
                                pscs[c][:, j * 512 : (j + 1) * 512],
                                lhsT=kt_t[et][jp, mt * 128 : (mt + 1) * 128],
                                rhs=qt_t[et][jp, cs],
                                start=True,
                                stop=True,
                            )
                    for c in range(2):
                        cs = slice(c * 512, (c + 1) * 512)
                        p2 = pbp.tile([128, 2, 512], bf16, name="p2")
                        nc.scalar.activation(
                            p2[:, :, :],
                            pscs[c][:, :].rearrange("p (j l) -> p j l", j=2),
                            Exp, scale=SCALE, bias=negC[:, :],
                        )
                        if c == 0 and avr_jobs:
                            avr_jobs.pop(0)()
                        nc.vector.tensor_mul(
                            p2[:, :, :],
                            p2[:, :, :],
                            mk_t[mt][:, cs].rearrange("p (o l) -> p o l", o=1).to_broadcast([128, 2, 512]),
                        )
                        pref[(et, r + c)] = p2


            # ---- output projection, overlapped with the final flush:
            # lt=0,1 accumulate p=0..6 first (ot_t[7] not ready yet), the
            # flush (which waits on the reciprocal) runs behind them, then
            # the p=7 matmuls complete those tiles.
            early = []
            for lt in range(2):
                ps = scp.tile([128, L], f32, tag="sc", name="yps")
                for fc in range(2):
                    cs = slice(fc * 512, (fc + 1) * 512)
                    for p in range(NT - 1):
                        for h in range(2):
                            nc.tensor.matmul(
                                ps[64 * h : 64 * (h + 1), cs],
                                lhsT=ot_t[p][
                                    :, lt * 128 + 64 * h : lt * 128 + 64 * (h + 1)
                                ],
                                rhs=wo_t[p][:, cs],
                                start=(p == 0),
                                stop=False,
                                tile_position=(0, 64 * h),
                            )
                early.append(ps)

            if pending is not None:
                rb_f = rbp.tile([128, L], bf16, name="rbf")
                flush_half(pending, 0, rb_f)
                flush_half(pending, 1, rb_f)
                pending = None

            for lt in range(NT):
                if lt < 2:
                    ps = early[lt]
                else:
                    ps = scp.tile([128, L], f32, tag="sc", name="yps")
                for fc in range(2):
                    cs = slice(fc * 512, (fc + 1) * 512)
                    ps_range = range(NT - 1, NT) if lt < 2 else range(NT)
                    for p in ps_range:
                        for h in range(2):
                            nc.tensor.matmul(
                                ps[64 * h : 64 * (h + 1), cs],
                                lhsT=ot_t[p][
                                    :, lt * 128 + 64 * h : lt * 128 + 64 * (h + 1)
                                ],
                                rhs=wo_t[p][:, cs],
                                start=(p == 0 and lt >= 2),
                                stop=(p == NT - 1),
                                tile_position=(0, 64 * h),
                            )
                y = ybp.tile([128, L], f16, name="y")
                # evict per 512-col half on alternating engines: each half
                # starts as soon as its fc accumulation stops, overlapping
                # the remaining matmuls and pipelining the final DMAs
                for fc in range(2):
                    cs = slice(fc * 512, (fc + 1) * 512)
                    if (2 * lt + fc) % 2 == 0:
                        nc.scalar.copy(out=y[:, cs], in_=ps[:, cs])
                    else:
                        nc.vector.tensor_copy(out=y[:, cs], in_=ps[:, cs])
                nc.sync.dma_start(out=out[lt * 128 : (lt + 1) * 128, :], in_=y[:, :])

    nc.finalize()
    return nc


def _get_nc():
    global _NC_CACHE
    if _NC_CACHE is None:
        _NC_CACHE = _build()
    return _NC_CACHE


def _make_in_maps(x, mask, Wk, Wv, Wq, Wo):
    f16 = np.float16
    bf16 = ml_dtypes.bfloat16
    wqT = np.ascontiguousarray(Wq.T).astype(f16)
    wkT = np.ascontiguousarray(Wk.T).astype(f16)
    wvT = np.ascontiguousarray(Wv.T).astype(bf16)
    woT = np.ascontiguousarray(Wo.T).astype(f16)
    maskT = np.ascontiguousarray(mask[0].T).astype(bf16)
    in_maps = []
    for b in range(N_CORES):
        in_maps.append(
            {
                "xT": np.ascontiguousarray(x[b].T).astype(f16),
                "wqT": wqT,
                "wkT": wkT,
                "wvT": wvT,
                "woT": woT,
                "maskT": maskT,
            }
        )
    return in_maps


def _run(x, mask, Wk, Wv, Wq, Wo, trace=False):
    from concourse.bass_utils import run_bass_kernel_spmd

    nc = _get_nc()
    in_maps = _make_in_maps(x, mask, Wk, Wv, Wq, Wo)
    res = run_bass_kernel_spmd(nc, in_maps, list(range(N_CORES)), trace=trace)
    y = np.stack([res.results[b]["out"] for b in range(N_CORES)], axis=0)
    return y.astype(np.float32), res


def kernel(x, mask, Wk, Wv, Wq, Wo):
    y, _ = _run(x, mask, Wk, Wv, Wq, Wo, trace=False)
    return y



# revision 15
# speedup vs baseline: 1.0346x; 1.0346x over previous
"""Trainium2 Bass kernel for batched multi-head attention.

HW exec time: ~264us full-clock (baseline 313us). Verified rel err 0.0037.

Distribution: pure data parallel - batch b -> NeuronCore b.

v5 vs v4 (275.5us): trailing AV/Z emitted first in each 2-step
group (ready work leads the PE FIFO; sem-waiting scores no longer
block it), TRAIL=6.

v4 vs v3 (278us):
  - 2-step grouping with same-shape matmul clustering: scores for both
    c-chunks of a key-tile emitted as one 4-MM same-shape burst, AV
    likewise; LDWEIGHTS pipelines within a same-shape burst (~224ns/slot
    vs ~302ns/slot mixed).
  - TRAIL=4 so the trailing AV/Z work aligns to (mt) pairs.
  - prefix shortened: V0,V1,K0,V2,V3,Q0 up front (first score at
    ~34us instead of ~48us); V4..V7 projected inside row 0 as 8-MM
    half-chunks in the 1-bank work pool; K1/Q1 fillers at r=8..14.

PSUM: scores 2x[128,1024] (4 banks), AV [128,1024] (2), Z [97,512] (1),
work [128,512] (1) = 8 banks.
"""

import sys

if "/opt/trn_rl_repo" not in sys.path:
    sys.path.insert(0, "/opt/trn_rl_repo")

import numpy as np
import ml_dtypes

B = 8
L = 1024
D = 1024
H = 16
DH = 64
NT = 8
SCALE = 0.125
EXP_SHIFT = -32.0
TRAIL = 6  # AV/Z trail the scores by this many steps (even: aligns mt pairs)
N_CORES = 8
NSTEP = 2 * NT

_NC_CACHE = None


def _build():
    import concourse.bass as bass  # noqa: F401
    from concourse import bacc, mybir
    import concourse.tile as tile

    f32 = mybir.dt.float32
    f16 = mybir.dt.float16
    bf16 = mybir.dt.bfloat16
    Exp = mybir.ActivationFunctionType.Exp

    nc = bacc.Bacc(None, target_bir_lowering=False)

    xT = nc.declare_dram_parameter("xT", [D, L], f16, isOutput=False)
    wqT = nc.declare_dram_parameter("wqT", [D, D], f16, isOutput=False)
    wkT = nc.declare_dram_parameter("wkT", [D, D], f16, isOutput=False)
    wvT = nc.declare_dram_parameter("wvT", [D, D], bf16, isOutput=False)
    woT = nc.declare_dram_parameter("woT", [D, D], f16, isOutput=False)
    mskT = nc.declare_dram_parameter("maskT", [L, L], bf16, isOutput=False)
    out = nc.declare_dram_parameter("out", [L, D], f16, isOutput=True)

    with tile.TileContext(nc) as tc:
        with (
            tc.tile_pool(name="big", bufs=1) as big,
            tc.tile_pool(name="pb", bufs=TRAIL + 3) as pbp,
            tc.tile_pool(name="sm", bufs=2) as smp,
            tc.tile_pool(name="rb", bufs=2) as rbp,
            tc.tile_pool(name="avr", bufs=2) as avrp,
            tc.tile_pool(name="yb", bufs=2) as ybp,
            tc.tile_pool(name="sc", bufs=2, space="PSUM") as scp,
            tc.tile_pool(name="pav", bufs=1, space="PSUM") as pavp,
            tc.tile_pool(name="pz", bufs=1, space="PSUM") as pzp,
            tc.tile_pool(name="wk", bufs=1, space="PSUM") as wkp,
        ):
            x_t = [big.tile([128, L], f16, tag=f"x{i}", name=f"x{i}") for i in range(NT)]
            wq_t = [big.tile([128, D], f16, tag=f"wq{i}", name=f"wq{i}") for i in range(NT)]
            wk_t = [big.tile([128, D], f16, tag=f"wk{i}", name=f"wk{i}") for i in range(NT)]
            wv_t = [big.tile([128, D], bf16, tag=f"wv{i}", name=f"wv{i}") for i in range(NT)]
            wo_t = [big.tile([128, D], f16, tag=f"wo{i}", name=f"wo{i}") for i in range(NT)]
            mk_t = [big.tile([128, L], bf16, tag=f"mk{i}", name=f"mk{i}") for i in range(NT)]
            qt_t = [big.tile([128, L], f16, tag=f"qt{i}", name=f"qt{i}") for i in range(NT)]
            kt_t = [big.tile([128, L], f16, tag=f"kt{i}", name=f"kt{i}") for i in range(NT)]
            v_t = [big.tile([128, H, DH], bf16, tag=f"v{i}", name=f"v{i}") for i in range(NT)]
            ot_t = [big.tile([128, L], f16, tag=f"ot{i}", name=f"ot{i}") for i in range(NT)]
            ones32 = big.tile([128, 32], bf16, tag="ones32")
            sel = big.tile([128, 4, 64], bf16, tag="sel")
            negC = big.tile([128, 1], f32, tag="negC")

            # input DMAs, ordered by first use and striped so early tiles
            # land first: x+wv quarters (V0 starts ~3us in), then the col-0
            # chunks of wk/wq (all K0/Q0 needs), mask tiles, then the rest.
            for i in range(NT):
                sl = slice(i * 128, (i + 1) * 128)
                nc.sync.dma_start(out=wv_t[i][:, :], in_=wvT[sl, :])
                nc.sync.dma_start(out=x_t[i][:, :], in_=xT[sl, :])
            for i in range(NT):
                sl = slice(i * 128, (i + 1) * 128)
                nc.sync.dma_start(out=wk_t[i][:, 0:128], in_=wkT[sl, 0:128])
                nc.sync.dma_start(out=wq_t[i][:, 0:128], in_=wqT[sl, 0:128])
            for i in range(NT):
                sl = slice(i * 128, (i + 1) * 128)
                nc.sync.dma_start(out=mk_t[i][:, :], in_=mskT[sl, :])
            for i in range(NT):
                sl = slice(i * 128, (i + 1) * 128)
                nc.sync.dma_start(out=wk_t[i][:, 128:1024], in_=wkT[sl, 128:1024])
                nc.sync.dma_start(out=wq_t[i][:, 128:1024], in_=wqT[sl, 128:1024])
            for i in range(NT):
                sl = slice(i * 128, (i + 1) * 128)
                nc.sync.dma_start(out=wo_t[i][:, :], in_=woT[sl, :])

            nc.vector.memset(ones32[:, :], 1.0)
            nc.vector.memset(sel[:, :, :], 0.0)
            for q in range(4):
                nc.vector.memset(sel[32 * q : 32 * q + 1, q, :], 1.0)
            nc.vector.memset(negC[:, :], EXP_SHIFT)

            def proj_full(lhs_fn, rhs_fn, evict):
                # col-tiled (128x64 mode): two concurrent 64-col tiles per
                # 512-col step; tiled rounds chain with ~0 gap vs 77ns/MM in
                # full 128x128 mode.
                ps = scp.tile([128, L], f32, tag="sc", name="projps")
                for c in range(2):
                    cs = slice(c * 512, (c + 1) * 512)
                    for dt in range(NT):
                        for h in range(2):
                            nc.tensor.matmul(
                                ps[64 * h : 64 * (h + 1), cs],
                                lhsT=lhs_fn(dt, h),
                                rhs=rhs_fn(dt, cs),
                                start=(dt == 0),
                                stop=(dt == NT - 1),
                                tile_position=(0, 64 * h),
                            )
                evict(ps)

            def v_proj(mt):
                proj_full(
                    lambda dt, h: x_t[dt][:, mt * 128 + 64 * h : mt * 128 + 64 * (h + 1)],
                    lambda dt, cs: wv_t[dt][:, cs],
                    lambda ps: nc.vector.tensor_copy(
                        out=v_t[mt][:, :, :],
                        in_=ps[:, :].rearrange("p (h e) -> p h e", h=H),
                    ),
                )

            def kq_proj(w_tiles, dst, et):
                proj_full(
                    lambda dt, h: w_tiles[dt][:, et * 128 + 64 * h : et * 128 + 64 * (h + 1)],
                    lambda dt, cs: x_t[dt][:, cs],
                    lambda ps: nc.vector.tensor_copy(out=dst[et][:, :], in_=ps[:, :]),
                )

            def kq_proj_half(w_tiles, dst, et, c):
                ps = wkp.tile([128, 512], f32, tag="wk", name="hp")
                cs = slice(c * 512, (c + 1) * 512)
                for dt in range(NT):
                    for h in range(2):
                        nc.tensor.matmul(
                            ps[64 * h : 64 * (h + 1), :],
                            lhsT=w_tiles[dt][
                                :, et * 128 + 64 * h : et * 128 + 64 * (h + 1)
                            ],
                            rhs=x_t[dt][:, cs],
                            start=(dt == 0),
                            stop=(dt == NT - 1),
                            tile_position=(0, 64 * h),
                        )
                nc.vector.tensor_copy(out=dst[et][:, cs], in_=ps[:, :])

            def v_proj_half(mt, c):
                # half of V proj: output heads c*8..c*8+8 of v_t[mt]
                ps = wkp.tile([128, 512], f32, tag="wk", name="vh")
                cs = slice(c * 512, (c + 1) * 512)
                for dt in range(NT):
                    for h in range(2):
                        nc.tensor.matmul(
                            ps[64 * h : 64 * (h + 1), :],
                            lhsT=x_t[dt][
                                :, mt * 128 + 64 * h : mt * 128 + 64 * (h + 1)
                            ],
                            rhs=wv_t[dt][:, cs],
                            start=(dt == 0),
                            stop=(dt == NT - 1),
                            tile_position=(0, 64 * h),
                        )
                nc.vector.tensor_copy(
                    out=v_t[mt][:, c * 8 : (c + 1) * 8, :],
                    in_=ps[:, :].rearrange("p (h e) -> p h e", h=8),
                )

            # ---- prefix: V0,V1,K0,V2,V3,Q0 (V4..V7 run inside row 0)
            v_proj(0)
            v_proj(1)
            kq_proj(wk_t, kt_t, 0)
            v_proj(2)
            v_proj(3)
            kq_proj(wq_t, qt_t, 0)

            # ---- head: 64 groups of 2 steps: (et, mt) with c=0,1
            def flush_half(p, c, rb):
                # broadcast reciprocal rows via sel matrices in the same
                # (128,64) col-tiled mode as AV/fillers (no PE mode switch)
                pet, pavr, przb = p
                cs = slice(c * 512, (c + 1) * 512)
                psr = wkp.tile([128, 512], f32, tag="wk", name="psr")
                for j in range(2):
                    q = 2 * j + c
                    nc.tensor.matmul(
                        psr[64 * j : 64 * j + 64, :],
                        lhsT=sel[:, q, :],
                        rhs=przb[:, :],
                        start=True,
                        stop=True,
                        tile_position=(0, 64 * j),
                    )
                nc.vector.tensor_copy(out=rb[:, cs], in_=psr[:, :])
                if c == 1:
                    nc.vector.tensor_mul(ot_t[pet][:, :], pavr[:, :], rb[:, :])

            pending = None
            flushing = None
            rb_cur = None
            av = None
            zt = None
            avr_jobs = []
            deferred_av = None
            pref = {}
            NSTEPS = NT * NT * 2

            # ---- HAM warmup: ~4us of dummy matmuls on resident data while
            # the input DMAs stream, so the PE clock-gate opens (1.2->2.4GHz)
            # before the first real projection.
            wps = wkp.tile([128, 512], f32, tag="wk", name="warm")
            for wi in range(20):
                nc.tensor.matmul(
                    wps[0:64, 0:256],
                    lhsT=sel[:, 0, :],
                    rhs=sel[:, :, :].rearrange("p a b -> p (a b)"),
                    start=(wi == 0),
                    stop=(wi == 19),
                    tile_position=(0, 0),
                )
            # per-group emission: [Z (128x32)] [AV+flush+fillers (128x64)]
            # [scores (64x128)] — 3 mode switches/group, and scores go last
            # so the PE has a full group of work queued while waiting for
            # exp(g-1) to free the scores PSUM pool.
            for s2 in range(0, NSTEPS + TRAIL, 2):
                as2 = s2 - TRAIL
                if as2 >= 0:
                    aet, ar = divmod(as2, NSTEP)
                    amt = ar // 2
                    if ar == 0:
                        av = pavp.tile([128, L], f32, tag="av", name="av")
                        zt = pzp.tile([128, 512], f32, tag="z", name="z")
                    p2c = [pref.pop((aet, ar)), pref.pop((aet, ar + 1))]
                    for j in range(2):
                        for cc in range(2):
                            q = 2 * j + cc
                            nc.tensor.matmul(
                                zt[32 * q : 32 * q + 32, :],
                                lhsT=ones32[:, :],
                                rhs=p2c[cc][:, j, :],
                                start=(amt == 0),
                                stop=(amt == NT - 1),
                                tile_position=(0, 32 * q),
                            )

                    def emit_av(amt=amt, aet=aet, p2c=p2c, av=av):
                        for c in range(2):
                            acs = slice(c * 512, (c + 1) * 512)
                            for j in range(2):
                                nc.tensor.matmul(
                                    av[j * 64 : (j + 1) * 64, acs],
                                    lhsT=v_t[amt][:, 2 * aet + j, :],
                                    rhs=p2c[c][:, j, :],
                                    start=(amt == 0),
                                    stop=(amt == NT - 1),
                                    tile_position=(0, 64 * j),
                                )

                    # at row starts the av banks are freed by the avr copy
                    # only ~1.5us into the group: emit AV after the fillers
                    # so the PE doesn't stall on the bank WAR
                    if ar == 0 and s2 < NSTEPS:
                        deferred_av = emit_av
                    else:
                        emit_av()
                    if ar == NSTEP - 2:
                        # avr eviction: two 512-col halves slotted between
                        # the exps of this and the next group (each half
                        # fits the exp-chain slack; a full copy would delay
                        # exp c1 enough to stall the next group's scores)
                        avr = avrp.tile([128, L], bf16, tag="avr", name="avr")
                        if s2 >= NSTEPS - 2:
                            nc.scalar.copy(out=avr[:, :], in_=av[:, :])
                        else:
                            for hc in range(2):
                                hs = slice(hc * 512, (hc + 1) * 512)
                                avr_jobs.append(
                                    lambda hs=hs, avr=avr, av=av: nc.scalar.copy(
                                        out=avr[:, hs], in_=av[:, hs]
                                    )
                                )
                        rz = smp.tile([128, 512], f32, tag="rz", name="rz")
                        nc.vector.reciprocal_approx_fast(out=rz[:, :], in_=zt[:, :])
                        rzb = smp.tile([128, 512], bf16, tag="rzb", name="rzb")
                        nc.vector.tensor_copy(out=rzb[:, :], in_=rz[:, :])
                        pending = (aet, avr, rzb)
                if s2 < NSTEPS:
                    et, r = divmod(s2, NSTEP)
                    mt = r // 2
                    if r == 6 and pending is not None:
                        flushing = pending
                        pending = None
                        rb_cur = rbp.tile([128, L], bf16, name="rb")
                        flush_half(flushing, 0, rb_cur)
                    if r == 10 and flushing is not None:
                        flush_half(flushing, 1, rb_cur)
                        flushing = None
                    # fillers: row 0 hosts V4..V7 c0-halves (heads 0-7,
                    # all rows 0-3 need only these); c1-halves spread over
                    # rows 0-3 (first needed by row 4)
                    if et == 0 and r <= 6:
                        v_proj_half(4 + r // 2, 0)
                    if et == 0 and r == 8:
                        v_proj_half(4, 1)
                    if 1 <= et <= 3 and r == 0:
                        v_proj_half(4 + et, 1)
                    if r == 4 and et >= 1:
                        # current row's K c1 half (needed from r=8)
                        kq_proj_half(wk_t, kt_t, et, 1)
                    if et + 1 < NT:
                        if r == (8 if et >= 1 else 10):
                            kq_proj_half(wq_t, qt_t, et + 1, 0)
                        elif r == 12:
                            kq_proj_half(wq_t, qt_t, et + 1, 1)
                        elif r == 14:
                            kq_proj_half(wk_t, kt_t, et + 1, 0)
                    if deferred_av is not None:
                        deferred_av()
                        deferred_av = None
                    # scores last (64x128 row-tiled mode)
                    pscs = []
                    for c in range(2):
                        psc = scp.tile([128, L], f32, tag="sc", name="sps")
                        pscs.append(psc)
                    for c in range(2):
                        cs = slice(c * 512, (c + 1) * 512)
                        for j in range(2):
                            jp = slice(j * 64, (j + 1) * 64)
                            nc.tensor.matmul(
                                pscs[c][:, j * 512 : (j + 1) * 512],
                                lhsT=kt_t[et][jp, mt * 128 : (mt + 1) * 128],
                                rhs=qt_t[et][jp, cs],
                                start=True,
                                stop=True,
                            )
                    for c in range(2):
                        cs = slice(c * 512, (c + 1) * 512)
                        p2 = pbp.tile([128, 2, 512], bf16, name="p2")
                        nc.scalar.activation(
                            p2[:, :, :],
                            pscs[c][:, :].rearrange("p (j l) -> p j l", j=2),
                            Exp, scale=SCALE, bias=negC[:, :],
                        )
                        if c == 0 and avr_jobs:
                            avr_jobs.pop(0)()
                        nc.vector.tensor_mul(
                            p2[:, :, :],
                            p2[:, :, :],
                            mk_t[mt][:, cs].rearrange("p (o l) -> p o l", o=1).to_broadcast([128, 2, 512]),
                        )
                        pref[(et, r + c)] = p2


            # ---- output projection, overlapped with the final flush:
            # lt=0,1 accumulate p=0..6 first (ot_t[7] not ready yet), the
            # flush (which waits on the reciprocal) runs behind them, then
            # the p=7 matmuls complete those tiles.
            early = []
            for lt in range(2):
                ps = scp.tile([128, L], f32, tag="sc", name="yps")
                for fc in range(2):
                    cs = slice(fc * 512, (fc + 1) * 512)
                    for p in range(NT - 1):
                        for h in range(2):
                            nc.tensor.matmul(
                                ps[64 * h : 64 * (h + 1), cs],
                                lhsT=ot_t[p][
                                    :, lt * 128 + 64 * h : lt * 128 + 64 * (h + 1)
                                ],
                                rhs=wo_t[p][:, cs],
                                start=(p == 0),
                                stop=False,
                                tile_position=(0, 64 * h),
                            )
                early.append(ps)

            if pending is not None:
                rb_f = rbp.tile([128, L], bf16, name="rbf")
                flush_half(pending, 0, rb_f)
                flush_half(pending, 1, rb_f)
                pending = None

            for lt in range(NT):
                if lt < 2:
                    ps = early[lt]
                else:
                    ps = scp.tile([128, L], f32, tag="sc", name="yps")
                for fc in range(2):
                    cs = slice(fc * 512, (fc + 1) * 512)
                    ps_range = range(NT - 1, NT) if lt < 2 else range(NT)
                    for p in ps_range:
                        for h in range(2):
                            nc.tensor.matmul(
                                ps[64 * h : 64 * (h + 1), cs],
                                lhsT=ot_t[p][
                                    :, lt * 128 + 64 * h : lt * 128 + 64 * (h + 1)
                                ],
                                rhs=wo_t[p][:, cs],
                                start=(p == 0 and lt >= 2),
                                stop=(p == NT - 1),
                                tile_position=(0, 64 * h),
                            )
                y = ybp.tile([128, L], f16, name="y")
                # alternate eviction engines so the tail drains in parallel
                if lt % 2 == 0:
                    nc.scalar.copy(out=y[:, :], in_=ps[:, :])
                else:
                    nc.vector.tensor_copy(out=y[:, :], in_=ps[:, :])
                nc.sync.dma_start(out=out[lt * 128 : (lt + 1) * 128, :], in_=y[:, :])

    nc.finalize()
    return nc


def _get_nc():
    global _NC_CACHE
    if _NC_CACHE is None:
        _NC_CACHE = _build()
    return _NC_CACHE


def _make_in_maps(x, mask, Wk, Wv, Wq, Wo):
    f16 = np.float16
    bf16 = ml_dtypes.bfloat16
    wqT = np.ascontiguousarray(Wq.T).astype(f16)
    wkT = np.ascontiguousarray(Wk.T).astype(f16)
    wvT = np.ascontiguousarray(Wv.T).astype(bf16)
    woT = np.ascontiguousarray(Wo.T).astype(f16)
    maskT = np.ascontiguousarray(mask[0].T).astype(bf16)
    in_maps = []
    for b in range(N_CORES):
        in_maps.append(
            {
                "xT": np.ascontiguousarray(x[b].T).astype(f16),
                "wqT": wqT,
                "wkT": wkT,
                "wvT": wvT,
                "woT": woT,
                "maskT": maskT,
            }
        )
    return in_maps


def _run(x, mask, Wk, Wv, Wq, Wo, trace=False):
    from concourse.bass_utils import run_bass_kernel_spmd

    nc = _get_nc()
    in_maps = _make_in_maps(x, mask, Wk, Wv, Wq, Wo)
    res = run_bass_kernel_spmd(nc, in_maps, list(range(N_CORES)), trace=trace)
    y = np.stack([res.results[b]["out"] for b in range(N_CORES)], axis=0)
    return y.astype(np.float32), res


def kernel(x, mask, Wk, Wv, Wq, Wo):
    y, _ = _run(x, mask, Wk, Wv, Wq, Wo, trace=False)
    return y



# revision 18
# speedup vs baseline: 1.0562x; 1.0208x over previous
"""Trainium2 Bass kernel for batched multi-head attention.

HW exec time: 262.6-265.0us full-clock across runs (v5 baseline 264us
full-clock / 313us throttled). Verified rel err 0.0037. v6: col-tiled
(128x64) projection pairs (kills the 77ns/MM full-mode weight-swap gap),
3 tiling-mode switches/group with scores emitted last, sel-matrix flush
sharing the AV mode, avr eviction split into two 512-col halves between
consecutive groups' exps, DMA ordering by first use, HAM warmup MMs.

Distribution: pure data parallel - batch b -> NeuronCore b.

v5 vs v4 (275.5us): trailing AV/Z emitted first in each 2-step
group (ready work leads the PE FIFO; sem-waiting scores no longer
block it), TRAIL=6.

v4 vs v3 (278us):
  - 2-step grouping with same-shape matmul clustering: scores for both
    c-chunks of a key-tile emitted as one 4-MM same-shape burst, AV
    likewise; LDWEIGHTS pipelines within a same-shape burst (~224ns/slot
    vs ~302ns/slot mixed).
  - TRAIL=4 so the trailing AV/Z work aligns to (mt) pairs.
  - prefix shortened: V0,V1,K0,V2,V3,Q0 up front (first score at
    ~34us instead of ~48us); V4..V7 projected inside row 0 as 8-MM
    half-chunks in the 1-bank work pool; K1/Q1 fillers at r=8..14.

PSUM: scores 2x[128,1024] (4 banks), AV [128,1024] (2), Z [97,512] (1),
work [128,512] (1) = 8 banks.
"""

import sys

if "/opt/trn_rl_repo" not in sys.path:
    sys.path.insert(0, "/opt/trn_rl_repo")

import numpy as np
import ml_dtypes

B = 8
L = 1024
D = 1024
H = 16
DH = 64
NT = 8
SCALE = 0.125
EXP_SHIFT = -32.0
TRAIL = 6  # AV/Z trail the scores by this many steps (even: aligns mt pairs)
N_CORES = 8
NSTEP = 2 * NT

_NC_CACHE = None


def _build():
    import concourse.bass as bass  # noqa: F401
    from concourse import bacc, mybir
    import concourse.tile as tile

    f32 = mybir.dt.float32
    f16 = mybir.dt.float16
    bf16 = mybir.dt.bfloat16
    Exp = mybir.ActivationFunctionType.Exp

    nc = bacc.Bacc(None, target_bir_lowering=False)

    xT = nc.declare_dram_parameter("xT", [D, L], f16, isOutput=False)
    wqT = nc.declare_dram_parameter("wqT", [D, D], f16, isOutput=False)
    wkT = nc.declare_dram_parameter("wkT", [D, D], f16, isOutput=False)
    wvT = nc.declare_dram_parameter("wvT", [D, D], bf16, isOutput=False)
    woT = nc.declare_dram_parameter("woT", [D, D], f16, isOutput=False)
    mskT = nc.declare_dram_parameter("maskT", [L, L], bf16, isOutput=False)
    out = nc.declare_dram_parameter("out", [L, D], f16, isOutput=True)

    with tile.TileContext(nc) as tc:
        with (
            tc.tile_pool(name="big", bufs=1) as big,
            tc.tile_pool(name="pb", bufs=TRAIL + 3) as pbp,
            tc.tile_pool(name="sm", bufs=2) as smp,
            tc.tile_pool(name="rb", bufs=2) as rbp,
            tc.tile_pool(name="avr", bufs=2) as avrp,
            tc.tile_pool(name="yb", bufs=2) as ybp,
            tc.tile_pool(name="sc", bufs=2, space="PSUM") as scp,
            tc.tile_pool(name="pav", bufs=1, space="PSUM") as pavp,
            tc.tile_pool(name="pz", bufs=1, space="PSUM") as pzp,
            tc.tile_pool(name="wk", bufs=1, space="PSUM") as wkp,
        ):
            x_t = [big.tile([128, L], f16, tag=f"x{i}", name=f"x{i}") for i in range(NT)]
            wq_t = [big.tile([128, D], f16, tag=f"wq{i}", name=f"wq{i}") for i in range(NT)]
            wk_t = [big.tile([128, D], f16, tag=f"wk{i}", name=f"wk{i}") for i in range(NT)]
            wv_t = [big.tile([128, D], bf16, tag=f"wv{i}", name=f"wv{i}") for i in range(NT)]
            wo_t = [big.tile([128, D], f16, tag=f"wo{i}", name=f"wo{i}") for i in range(NT)]
            mk_t = [big.tile([128, L], bf16, tag=f"mk{i}", name=f"mk{i}") for i in range(NT)]
            qt_t = [big.tile([128, L], f16, tag=f"qt{i}", name=f"qt{i}") for i in range(NT)]
            kt_t = [big.tile([128, L], f16, tag=f"kt{i}", name=f"kt{i}") for i in range(NT)]
            v_t = [big.tile([128, H, DH], bf16, tag=f"v{i}", name=f"v{i}") for i in range(NT)]
            ot_t = [big.tile([128, L], f16, tag=f"ot{i}", name=f"ot{i}") for i in range(NT)]
            ones32 = big.tile([128, 32], bf16, tag="ones32")
            sel = big.tile([128, 4, 64], bf16, tag="sel")
            negC = big.tile([128, 1], f32, tag="negC")

            # input DMAs, ordered by first use and striped so early tiles
            # land first: x+wv quarters (V0 starts ~3us in), then the col-0
            # chunks of wk/wq (all K0/Q0 needs), mask tiles, then the rest.
            for i in range(NT):
                sl = slice(i * 128, (i + 1) * 128)
                nc.sync.dma_start(out=wv_t[i][:, :], in_=wvT[sl, :])
                nc.sync.dma_start(out=x_t[i][:, :], in_=xT[sl, :])
            for i in range(NT):
                sl = slice(i * 128, (i + 1) * 128)
                nc.sync.dma_start(out=wk_t[i][:, 0:128], in_=wkT[sl, 0:128])
                nc.sync.dma_start(out=wq_t[i][:, 0:128], in_=wqT[sl, 0:128])
            for i in range(NT):
                sl = slice(i * 128, (i + 1) * 128)
                nc.sync.dma_start(out=mk_t[i][:, :], in_=mskT[sl, :])
            for i in range(NT):
                sl = slice(i * 128, (i + 1) * 128)
                nc.sync.dma_start(out=wk_t[i][:, 128:1024], in_=wkT[sl, 128:1024])
                nc.sync.dma_start(out=wq_t[i][:, 128:1024], in_=wqT[sl, 128:1024])
            for i in range(NT):
                sl = slice(i * 128, (i + 1) * 128)
                nc.sync.dma_start(out=wo_t[i][:, :], in_=woT[sl, :])

            nc.vector.memset(ones32[:, :], 1.0)
            nc.vector.memset(sel[:, :, :], 0.0)
            for q in range(4):
                nc.vector.memset(sel[32 * q : 32 * q + 1, q, :], 1.0)
            nc.vector.memset(negC[:, :], EXP_SHIFT)

            def proj_full(lhs_fn, rhs_fn, evict):
                # col-tiled (128x64 mode): two concurrent 64-col tiles per
                # 512-col step; tiled rounds chain with ~0 gap vs 77ns/MM in
                # full 128x128 mode.
                ps = scp.tile([128, L], f32, tag="sc", name="projps")
                for c in range(2):
                    cs = slice(c * 512, (c + 1) * 512)
                    for dt in range(NT):
                        for h in range(2):
                            nc.tensor.matmul(
                                ps[64 * h : 64 * (h + 1), cs],
                                lhsT=lhs_fn(dt, h),
                                rhs=rhs_fn(dt, cs),
                                start=(dt == 0),
                                stop=(dt == NT - 1),
                                tile_position=(0, 64 * h),
                            )
                evict(ps)

            def v_proj(mt):
                proj_full(
                    lambda dt, h: x_t[dt][:, mt * 128 + 64 * h : mt * 128 + 64 * (h + 1)],
                    lambda dt, cs: wv_t[dt][:, cs],
                    lambda ps: nc.vector.tensor_copy(
                        out=v_t[mt][:, :, :],
                        in_=ps[:, :].rearrange("p (h e) -> p h e", h=H),
                    ),
                )

            def kq_proj(w_tiles, dst, et):
                proj_full(
                    lambda dt, h: w_tiles[dt][:, et * 128 + 64 * h : et * 128 + 64 * (h + 1)],
                    lambda dt, cs: x_t[dt][:, cs],
                    lambda ps: nc.vector.tensor_copy(out=dst[et][:, :], in_=ps[:, :]),
                )

            hp_open = {}

            def kq_proj_quarter(w_tiles, dst, et, c, half):
                # 8-MM quarter of a kq projection half: spreads filler work
                # more uniformly over group slots so light groups don't
                # expose the scalar exp-chain lag as pre-scores PE stalls
                key = (id(w_tiles), et, c)
                cs = slice(c * 512, (c + 1) * 512)
                if half == 0:
                    hp_open[key] = wkp.tile([128, 512], f32, tag="wk", name="hp")
                ps = hp_open[key]
                for dt in range(4 * half, 4 * half + 4):
                    for h in range(2):
                        nc.tensor.matmul(
                            ps[64 * h : 64 * (h + 1), :],
                            lhsT=w_tiles[dt][
                                :, et * 128 + 64 * h : et * 128 + 64 * (h + 1)
                            ],
                            rhs=x_t[dt][:, cs],
                            start=(dt == 0),
                            stop=(dt == NT - 1),
                            tile_position=(0, 64 * h),
                        )
                if half == 1:
                    nc.vector.tensor_copy(out=dst[et][:, cs], in_=ps[:, :])
                    del hp_open[key]

            def kq_proj_half(w_tiles, dst, et, c):
                kq_proj_quarter(w_tiles, dst, et, c, 0)
                kq_proj_quarter(w_tiles, dst, et, c, 1)

            def v_proj_half(mt, c):
                # half of V proj: output heads c*8..c*8+8 of v_t[mt]
                ps = wkp.tile([128, 512], f32, tag="wk", name="vh")
                cs = slice(c * 512, (c + 1) * 512)
                for dt in range(NT):
                    for h in range(2):
                        nc.tensor.matmul(
                            ps[64 * h : 64 * (h + 1), :],
                            lhsT=x_t[dt][
                                :, mt * 128 + 64 * h : mt * 128 + 64 * (h + 1)
                            ],
                            rhs=wv_t[dt][:, cs],
                            start=(dt == 0),
                            stop=(dt == NT - 1),
                            tile_position=(0, 64 * h),
                        )
                nc.vector.tensor_copy(
                    out=v_t[mt][:, c * 8 : (c + 1) * 8, :],
                    in_=ps[:, :].rearrange("p (h e) -> p h e", h=8),
                )

            # ---- prefix: V0,V1,K0,V2,V3,Q0 (V4..V7 run inside row 0)
            v_proj(0)
            v_proj(1)
            kq_proj(wk_t, kt_t, 0)
            v_proj(2)
            v_proj(3)
            kq_proj(wq_t, qt_t, 0)

            # ---- head: 64 groups of 2 steps: (et, mt) with c=0,1
            def flush_half(p, c, rb):
                # broadcast reciprocal rows via sel matrices in the same
                # (128,64) col-tiled mode as AV/fillers (no PE mode switch)
                pet, pavr, przb = p
                cs = slice(c * 512, (c + 1) * 512)
                psr = wkp.tile([128, 512], f32, tag="wk", name="psr")
                for j in range(2):
                    q = 2 * j + c
                    nc.tensor.matmul(
                        psr[64 * j : 64 * j + 64, :],
                        lhsT=sel[:, q, :],
                        rhs=przb[:, :],
                        start=True,
                        stop=True,
                        tile_position=(0, 64 * j),
                    )
                nc.vector.tensor_copy(out=rb[:, cs], in_=psr[:, :])
                if c == 1:
                    nc.vector.tensor_mul(ot_t[pet][:, :], pavr[:, :], rb[:, :])

            pending = None
            flushing = None
            rb_cur = None
            av = None
            zt = None
            avr_jobs = []
            deferred_av = None
            pref = {}
            NSTEPS = NT * NT * 2

            # ---- HAM warmup: ~4us of dummy matmuls on resident data while
            # the input DMAs stream, so the PE clock-gate opens (1.2->2.4GHz)
            # before the first real projection.
            wps = wkp.tile([128, 512], f32, tag="wk", name="warm")
            for wi in range(20):
                nc.tensor.matmul(
                    wps[0:64, 0:256],
                    lhsT=sel[:, 0, :],
                    rhs=sel[:, :, :].rearrange("p a b -> p (a b)"),
                    start=(wi == 0),
                    stop=(wi == 19),
                    tile_position=(0, 0),
                )
            # per-group emission: [Z (128x32)] [AV+flush+fillers (128x64)]
            # [scores (64x128)] — 3 mode switches/group, and scores go last
            # so the PE has a full group of work queued while waiting for
            # exp(g-1) to free the scores PSUM pool.
            for s2 in range(0, NSTEPS + TRAIL, 2):
                as2 = s2 - TRAIL
                if as2 >= 0:
                    aet, ar = divmod(as2, NSTEP)
                    amt = ar // 2
                    if ar == 0:
                        av = pavp.tile([128, L], f32, tag="av", name="av")
                        zt = pzp.tile([128, 512], f32, tag="z", name="z")
                    p2c = [pref.pop((aet, ar)), pref.pop((aet, ar + 1))]
                    for j in range(2):
                        for cc in range(2):
                            q = 2 * j + cc
                            nc.tensor.matmul(
                                zt[32 * q : 32 * q + 32, :],
                                lhsT=ones32[:, :],
                                rhs=p2c[cc][:, j, :],
                                start=(amt == 0),
                                stop=(amt == NT - 1),
                                tile_position=(0, 32 * q),
                            )

                    def emit_av(amt=amt, aet=aet, p2c=p2c, av=av):
                        for c in range(2):
                            acs = slice(c * 512, (c + 1) * 512)
                            for j in range(2):
                                nc.tensor.matmul(
                                    av[j * 64 : (j + 1) * 64, acs],
                                    lhsT=v_t[amt][:, 2 * aet + j, :],
                                    rhs=p2c[c][:, j, :],
                                    start=(amt == 0),
                                    stop=(amt == NT - 1),
                                    tile_position=(0, 64 * j),
                                )

                    # at row starts the av banks are freed by the avr copy
                    # only ~1.5us into the group: emit AV after the fillers
                    # so the PE doesn't stall on the bank WAR
                    if ar == 0 and s2 < NSTEPS:
                        deferred_av = emit_av
                    else:
                        emit_av()
                    if ar == NSTEP - 2:
                        # avr eviction: two 512-col halves slotted between
                        # the exps of this and the next group (each half
                        # fits the exp-chain slack; a full copy would delay
                        # exp c1 enough to stall the next group's scores)
                        avr = avrp.tile([128, L], bf16, tag="avr", name="avr")
                        if s2 >= NSTEPS - 2:
                            nc.scalar.copy(out=avr[:, :], in_=av[:, :])
                        else:
                            for hc in range(2):
                                hs = slice(hc * 512, (hc + 1) * 512)
                                avr_jobs.append(
                                    lambda hs=hs, avr=avr, av=av: nc.scalar.copy(
                                        out=avr[:, hs], in_=av[:, hs]
                                    )
                                )
                        rz = smp.tile([128, 512], f32, tag="rz", name="rz")
                        nc.vector.reciprocal_approx_fast(out=rz[:, :], in_=zt[:, :])
                        rzb = smp.tile([128, 512], bf16, tag="rzb", name="rzb")
                        nc.vector.tensor_copy(out=rzb[:, :], in_=rz[:, :])
                        pending = (aet, avr, rzb)
                if s2 < NSTEPS:
                    et, r = divmod(s2, NSTEP)
                    mt = r // 2
                    if r == 8 and pending is not None:
                        flushing = pending
                        pending = None
                        rb_cur = rbp.tile([128, L], bf16, name="rb")
                        flush_half(flushing, 0, rb_cur)
                    if r == 10 and flushing is not None:
                        flush_half(flushing, 1, rb_cur)
                        flushing = None
                    # fillers: row 0 hosts V4..V7 c0-halves (heads 0-7,
                    # all rows 0-3 need only these); c1-halves spread over
                    # rows 0-3 (first needed by row 4)
                    if et == 0 and r <= 6:
                        v_proj_half(4 + r // 2, 0)
                    if et == 0 and r == 8:
                        v_proj_half(4, 1)
                    if 1 <= et <= 3 and r == 0:
                        v_proj_half(4 + et, 1)
                    if et >= 1:
                        # K c1 for this row: rows 1-3 burst it at r=2 (r=0
                        # holds their V half); rows 4-7 spread quarters over
                        # r=0/r=2 so no slot is left empty before scores
                        if et <= 3:
                            if r == 2:
                                kq_proj_half(wk_t, kt_t, et, 1)
                        elif r in (0, 2):
                            kq_proj_quarter(wk_t, kt_t, et, 1, r // 2)
                    if et + 1 < NT:
                        if et >= 1 and r in (4, 6):
                            kq_proj_quarter(wq_t, qt_t, et + 1, 0, (r - 4) // 2)
                        elif et == 0 and r == 10:
                            kq_proj_half(wq_t, qt_t, 1, 0)
                        elif r == 12:
                            kq_proj_half(wq_t, qt_t, et + 1, 1)
                        elif r == 14:
                            kq_proj_half(wk_t, kt_t, et + 1, 0)
                    if deferred_av is not None:
                        deferred_av()
                        deferred_av = None
                    # scores last (64x128 row-tiled mode)
                    pscs = []
                    for c in range(2):
                        psc = scp.tile([128, L], f32, tag="sc", name="sps")
                        pscs.append(psc)
                    for c in range(2):
                        cs = slice(c * 512, (c + 1) * 512)
                        for j in range(2):
                            jp = slice(j * 64, (j + 1) * 64)
                            nc.tensor.matmul(
                                pscs[c][:, j * 512 : (j + 1) * 512],
                                lhsT=kt_t[et][jp, mt * 128 : (mt + 1) * 128],
                                rhs=qt_t[et][jp, cs],
                                start=True,
                                stop=True,
                            )
                    for c in range(2):
                        cs = slice(c * 512, (c + 1) * 512)
                        p2 = pbp.tile([128, 2, 512], bf16, name="p2")
                        nc.scalar.activation(
                            p2[:, :, :],
                            pscs[c][:, :].rearrange("p (j l) -> p j l", j=2),
                            Exp, scale=SCALE, bias=negC[:, :],
                        )
                        if c == 0 and avr_jobs:
                            avr_jobs.pop(0)()
                        nc.vector.tensor_mul(
                            p2[:, :, :],
                            p2[:, :, :],
                            mk_t[mt][:, cs].rearrange("p (o l) -> p o l", o=1).to_broadcast([128, 2, 512]),
                        )
                        pref[(et, r + c)] = p2


            # ---- output projection, overlapped with the final flush:
            # lt=0,1 accumulate p=0..6 first (ot_t[7] not ready yet), the
            # flush (which waits on the reciprocal) runs behind them, then
            # the p=7 matmuls complete those tiles.
            early = []
            for lt in range(2):
                ps = scp.tile([128, L], f32, tag="sc", name="yps")
                for fc in range(2):
                    cs = slice(fc * 512, (fc + 1) * 512)
                    for p in range(NT - 1):
                        for h in range(2):
                            nc.tensor.matmul(
                                ps[64 * h : 64 * (h + 1), cs],
                                lhsT=ot_t[p][
                                    :, lt * 128 + 64 * h : lt * 128 + 64 * (h + 1)
                                ],
                                rhs=wo_t[p][:, cs],
                                start=(p == 0),
                                stop=False,
                                tile_position=(0, 64 * h),
                            )
                early.append(ps)

            if pending is not None:
                rb_f = rbp.tile([128, L], bf16, name="rbf")
                flush_half(pending, 0, rb_f)
                flush_half(pending, 1, rb_f)
                pending = None

            for lt in range(NT):
                if lt < 2:
                    ps = early[lt]
                else:
                    ps = scp.tile([128, L], f32, tag="sc", name="yps")
                for fc in range(2):
                    cs = slice(fc * 512, (fc + 1) * 512)
                    ps_range = range(NT - 1, NT) if lt < 2 else range(NT)
                    for p in ps_range:
                        for h in range(2):
                            nc.tensor.matmul(
                                ps[64 * h : 64 * (h + 1), cs],
                                lhsT=ot_t[p][
                                    :, lt * 128 + 64 * h : lt * 128 + 64 * (h + 1)
                                ],
                                rhs=wo_t[p][:, cs],
                                start=(p == 0 and lt >= 2),
                                stop=(p == NT - 1),
                                tile_position=(0, 64 * h),
                            )
                y = ybp.tile([128, L], f16, name="y")
                # alternate eviction engines so the tail drains in parallel
                if lt % 2 == 0:
                    nc.scalar.copy(out=y[:, :], in_=ps[:, :])
                else:
                    nc.vector.tensor_copy(out=y[:, :], in_=ps[:, :])
                nc.sync.dma_start(out=out[lt * 128 : (lt + 1) * 128, :], in_=y[:, :])

    nc.finalize()
    return nc


def _get_nc():
    global _NC_CACHE
    if _NC_CACHE is None:
        _NC_CACHE = _build()
    return _NC_CACHE


def _make_in_maps(x, mask, Wk, Wv, Wq, Wo):
    f16 = np.float16
    bf16 = ml_dtypes.bfloat16
    wqT = np.ascontiguousarray(Wq.T).astype(f16)
    wkT = np.ascontiguousarray(Wk.T).astype(f16)
    wvT = np.ascontiguousarray(Wv.T).astype(bf16)
    woT = np.ascontiguousarray(Wo.T).astype(f16)
    maskT = np.ascontiguousarray(mask[0].T).astype(bf16)
    in_maps = []
    for b in range(N_CORES):
        in_maps.append(
            {
                "xT": np.ascontiguousarray(x[b].T).astype(f16),
                "wqT": wqT,
                "wkT": wkT,
                "wvT": wvT,
                "woT": woT,
                "maskT": maskT,
            }
        )
    return in_maps


def _run(x, mask, Wk, Wv, Wq, Wo, trace=False):
    from concourse.bass_utils import run_bass_kernel_spmd

    nc = _get_nc()
    in_maps = _make_in_maps(x, mask, Wk, Wv, Wq, Wo)
    res = run_bass_kernel_spmd(nc, in_maps, list(range(N_CORES)), trace=trace)
    y = np.stack([res.results[b]["out"] for b in range(N_CORES)], axis=0)
    return y.astype(np.float32), res


def kernel(x, mask, Wk, Wv, Wq, Wo):
    y, _ = _run(x, mask, Wk, Wv, Wq, Wo, trace=False)
    return y



# revision 19
# speedup vs baseline: 1.0704x; 1.0135x over previous
"""Trainium2 Bass kernel for batched multi-head attention.

HW exec time: 262.6-265.0us full-clock across runs (v5 baseline 264us
full-clock / 313us throttled). Verified rel err 0.0037. v6: col-tiled
(128x64) projection pairs (kills the 77ns/MM full-mode weight-swap gap),
3 tiling-mode switches/group with scores emitted last, sel-matrix flush
sharing the AV mode, avr eviction split into two 512-col halves between
consecutive groups' exps, DMA ordering by first use, HAM warmup MMs.

Distribution: pure data parallel - batch b -> NeuronCore b.

v5 vs v4 (275.5us): trailing AV/Z emitted first in each 2-step
group (ready work leads the PE FIFO; sem-waiting scores no longer
block it), TRAIL=6.

v4 vs v3 (278us):
  - 2-step grouping with same-shape matmul clustering: scores for both
    c-chunks of a key-tile emitted as one 4-MM same-shape burst, AV
    likewise; LDWEIGHTS pipelines within a same-shape burst (~224ns/slot
    vs ~302ns/slot mixed).
  - TRAIL=4 so the trailing AV/Z work aligns to (mt) pairs.
  - prefix shortened: V0,V1,K0,V2,V3,Q0 up front (first score at
    ~34us instead of ~48us); V4..V7 projected inside row 0 as 8-MM
    half-chunks in the 1-bank work pool; K1/Q1 fillers at r=8..14.

PSUM: scores 2x[128,1024] (4 banks), AV [128,1024] (2), Z [97,512] (1),
work [128,512] (1) = 8 banks.
"""

import sys

if "/opt/trn_rl_repo" not in sys.path:
    sys.path.insert(0, "/opt/trn_rl_repo")

import numpy as np
import ml_dtypes

B = 8
L = 1024
D = 1024
H = 16
DH = 64
NT = 8
SCALE = 0.125
EXP_SHIFT = -32.0
TRAIL = 6  # AV/Z trail the scores by this many steps (even: aligns mt pairs)
N_CORES = 8
NSTEP = 2 * NT

_NC_CACHE = None


def _build():
    import concourse.bass as bass  # noqa: F401
    from concourse import bacc, mybir
    import concourse.tile as tile

    f32 = mybir.dt.float32
    f16 = mybir.dt.float16
    bf16 = mybir.dt.bfloat16
    Exp = mybir.ActivationFunctionType.Exp

    nc = bacc.Bacc(None, target_bir_lowering=False)

    xT = nc.declare_dram_parameter("xT", [D, L], f16, isOutput=False)
    wqT = nc.declare_dram_parameter("wqT", [D, D], f16, isOutput=False)
    wkT = nc.declare_dram_parameter("wkT", [D, D], f16, isOutput=False)
    wvT = nc.declare_dram_parameter("wvT", [D, D], bf16, isOutput=False)
    woT = nc.declare_dram_parameter("woT", [D, D], f16, isOutput=False)
    mskT = nc.declare_dram_parameter("maskT", [L, L], bf16, isOutput=False)
    out = nc.declare_dram_parameter("out", [L, D], f16, isOutput=True)

    with tile.TileContext(nc) as tc:
        with (
            tc.tile_pool(name="big", bufs=1) as big,
            tc.tile_pool(name="pb", bufs=TRAIL + 3) as pbp,
            tc.tile_pool(name="sm", bufs=2) as smp,
            tc.tile_pool(name="rb", bufs=2) as rbp,
            tc.tile_pool(name="avr", bufs=2) as avrp,
            tc.tile_pool(name="yb", bufs=2) as ybp,
            tc.tile_pool(name="sc", bufs=2, space="PSUM") as scp,
            tc.tile_pool(name="pav", bufs=1, space="PSUM") as pavp,
            tc.tile_pool(name="pz", bufs=1, space="PSUM") as pzp,
            tc.tile_pool(name="wk", bufs=1, space="PSUM") as wkp,
        ):
            x_t = [big.tile([128, L], f16, tag=f"x{i}", name=f"x{i}") for i in range(NT)]
            wq_t = [big.tile([128, D], f16, tag=f"wq{i}", name=f"wq{i}") for i in range(NT)]
            wk_t = [big.tile([128, D], f16, tag=f"wk{i}", name=f"wk{i}") for i in range(NT)]
            wv_t = [big.tile([128, D], bf16, tag=f"wv{i}", name=f"wv{i}") for i in range(NT)]
            wo_t = [big.tile([128, D], f16, tag=f"wo{i}", name=f"wo{i}") for i in range(NT)]
            mk_t = [big.tile([128, L], bf16, tag=f"mk{i}", name=f"mk{i}") for i in range(NT)]
            qt_t = [big.tile([128, L], f16, tag=f"qt{i}", name=f"qt{i}") for i in range(NT)]
            kt_t = [big.tile([128, L], f16, tag=f"kt{i}", name=f"kt{i}") for i in range(NT)]
            v_t = [big.tile([128, H, DH], bf16, tag=f"v{i}", name=f"v{i}") for i in range(NT)]
            ot_t = [big.tile([128, L], f16, tag=f"ot{i}", name=f"ot{i}") for i in range(NT)]
            ones32 = big.tile([128, 32], bf16, tag="ones32")
            sel = big.tile([128, 4, 64], bf16, tag="sel")
            negC = big.tile([128, 1], f32, tag="negC")

            # input DMAs, ordered by first use and striped so early tiles
            # land first: x+wv quarters (V0 starts ~3us in), then the col-0
            # chunks of wk/wq (all K0/Q0 needs), mask tiles, then the rest.
            for i in range(NT):
                sl = slice(i * 128, (i + 1) * 128)
                nc.sync.dma_start(out=wv_t[i][:, :], in_=wvT[sl, :])
                nc.sync.dma_start(out=x_t[i][:, :], in_=xT[sl, :])
            for i in range(NT):
                sl = slice(i * 128, (i + 1) * 128)
                nc.sync.dma_start(out=wk_t[i][:, 0:128], in_=wkT[sl, 0:128])
                nc.sync.dma_start(out=wq_t[i][:, 0:128], in_=wqT[sl, 0:128])
            for i in range(NT):
                sl = slice(i * 128, (i + 1) * 128)
                nc.sync.dma_start(out=mk_t[i][:, :], in_=mskT[sl, :])
            for i in range(NT):
                sl = slice(i * 128, (i + 1) * 128)
                nc.sync.dma_start(out=wk_t[i][:, 128:1024], in_=wkT[sl, 128:1024])
                nc.sync.dma_start(out=wq_t[i][:, 128:1024], in_=wqT[sl, 128:1024])
            for i in range(NT):
                sl = slice(i * 128, (i + 1) * 128)
                nc.sync.dma_start(out=wo_t[i][:, :], in_=woT[sl, :])

            nc.vector.memset(ones32[:, :], 1.0)
            nc.vector.memset(sel[:, :, :], 0.0)
            for q in range(4):
                nc.vector.memset(sel[32 * q : 32 * q + 1, q, :], 1.0)
            nc.vector.memset(negC[:, :], EXP_SHIFT)

            def proj_full(lhs_fn, rhs_fn, evict):
                # col-tiled (128x64 mode): two concurrent 64-col tiles per
                # 512-col step; tiled rounds chain with ~0 gap vs 77ns/MM in
                # full 128x128 mode.
                ps = scp.tile([128, L], f32, tag="sc", name="projps")
                for c in range(2):
                    cs = slice(c * 512, (c + 1) * 512)
                    for dt in range(NT):
                        for h in range(2):
                            nc.tensor.matmul(
                                ps[64 * h : 64 * (h + 1), cs],
                                lhsT=lhs_fn(dt, h),
                                rhs=rhs_fn(dt, cs),
                                start=(dt == 0),
                                stop=(dt == NT - 1),
                                tile_position=(0, 64 * h),
                            )
                evict(ps)

            def v_proj(mt):
                proj_full(
                    lambda dt, h: x_t[dt][:, mt * 128 + 64 * h : mt * 128 + 64 * (h + 1)],
                    lambda dt, cs: wv_t[dt][:, cs],
                    lambda ps: nc.vector.tensor_copy(
                        out=v_t[mt][:, :, :],
                        in_=ps[:, :].rearrange("p (h e) -> p h e", h=H),
                    ),
                )

            def kq_proj(w_tiles, dst, et):
                proj_full(
                    lambda dt, h: w_tiles[dt][:, et * 128 + 64 * h : et * 128 + 64 * (h + 1)],
                    lambda dt, cs: x_t[dt][:, cs],
                    lambda ps: nc.vector.tensor_copy(out=dst[et][:, :], in_=ps[:, :]),
                )

            hp_open = {}

            def kq_proj_quarter(w_tiles, dst, et, c, half):
                # 8-MM quarter of a kq projection half: spreads filler work
                # more uniformly over group slots so light groups don't
                # expose the scalar exp-chain lag as pre-scores PE stalls
                key = (id(w_tiles), et, c)
                cs = slice(c * 512, (c + 1) * 512)
                if half == 0:
                    hp_open[key] = wkp.tile([128, 512], f32, tag="wk", name="hp")
                ps = hp_open[key]
                for dt in range(4 * half, 4 * half + 4):
                    for h in range(2):
                        nc.tensor.matmul(
                            ps[64 * h : 64 * (h + 1), :],
                            lhsT=w_tiles[dt][
                                :, et * 128 + 64 * h : et * 128 + 64 * (h + 1)
                            ],
                            rhs=x_t[dt][:, cs],
                            start=(dt == 0),
                            stop=(dt == NT - 1),
                            tile_position=(0, 64 * h),
                        )
                if half == 1:
                    nc.vector.tensor_copy(out=dst[et][:, cs], in_=ps[:, :])
                    del hp_open[key]

            def kq_proj_half(w_tiles, dst, et, c):
                kq_proj_quarter(w_tiles, dst, et, c, 0)
                kq_proj_quarter(w_tiles, dst, et, c, 1)

            def v_proj_half(mt, c):
                # half of V proj: output heads c*8..c*8+8 of v_t[mt]
                ps = wkp.tile([128, 512], f32, tag="wk", name="vh")
                cs = slice(c * 512, (c + 1) * 512)
                for dt in range(NT):
                    for h in range(2):
                        nc.tensor.matmul(
                            ps[64 * h : 64 * (h + 1), :],
                            lhsT=x_t[dt][
                                :, mt * 128 + 64 * h : mt * 128 + 64 * (h + 1)
                            ],
                            rhs=wv_t[dt][:, cs],
                            start=(dt == 0),
                            stop=(dt == NT - 1),
                            tile_position=(0, 64 * h),
                        )
                nc.vector.tensor_copy(
                    out=v_t[mt][:, c * 8 : (c + 1) * 8, :],
                    in_=ps[:, :].rearrange("p (h e) -> p h e", h=8),
                )

            # ---- prefix: V0,V1,K0,V2,V3,Q0 (V4..V7 run inside row 0)
            v_proj(0)
            v_proj(1)
            kq_proj(wk_t, kt_t, 0)
            v_proj(2)
            v_proj(3)
            kq_proj(wq_t, qt_t, 0)

            # ---- head: 64 groups of 2 steps: (et, mt) with c=0,1
            def flush_half(p, c, rb):
                # broadcast reciprocal rows via sel matrices in the same
                # (128,64) col-tiled mode as AV/fillers (no PE mode switch)
                pet, pavr, przb = p
                cs = slice(c * 512, (c + 1) * 512)
                psr = wkp.tile([128, 512], f32, tag="wk", name="psr")
                for j in range(2):
                    q = 2 * j + c
                    nc.tensor.matmul(
                        psr[64 * j : 64 * j + 64, :],
                        lhsT=sel[:, q, :],
                        rhs=przb[:, :],
                        start=True,
                        stop=True,
                        tile_position=(0, 64 * j),
                    )
                nc.vector.tensor_copy(out=rb[:, cs], in_=psr[:, :])
                if c == 1:
                    nc.vector.tensor_mul(ot_t[pet][:, :], pavr[:, :], rb[:, :])

            pending = None
            flushing = None
            rb_cur = None
            av = None
            zt = None
            avr_jobs = []
            deferred_av = None
            pref = {}
            NSTEPS = NT * NT * 2

            # ---- HAM warmup: ~4us of dummy matmuls on resident data while
            # the input DMAs stream, so the PE clock-gate opens (1.2->2.4GHz)
            # before the first real projection.
            wps = wkp.tile([128, 512], f32, tag="wk", name="warm")
            for wi in range(20):
                nc.tensor.matmul(
                    wps[0:64, 0:256],
                    lhsT=sel[:, 0, :],
                    rhs=sel[:, :, :].rearrange("p a b -> p (a b)"),
                    start=(wi == 0),
                    stop=(wi == 19),
                    tile_position=(0, 0),
                )
            # per-group emission: [Z (128x32)] [AV+flush+fillers (128x64)]
            # [scores (64x128)] — 3 mode switches/group, and scores go last
            # so the PE has a full group of work queued while waiting for
            # exp(g-1) to free the scores PSUM pool.
            for s2 in range(0, NSTEPS + TRAIL, 2):
                as2 = s2 - TRAIL
                if as2 >= 0:
                    aet, ar = divmod(as2, NSTEP)
                    amt = ar // 2
                    if ar == 0:
                        av = pavp.tile([128, L], f32, tag="av", name="av")
                        zt = pzp.tile([128, 512], f32, tag="z", name="z")
                    p2c = [pref.pop((aet, ar)), pref.pop((aet, ar + 1))]
                    for j in range(2):
                        for cc in range(2):
                            q = 2 * j + cc
                            nc.tensor.matmul(
                                zt[32 * q : 32 * q + 32, :],
                                lhsT=ones32[:, :],
                                rhs=p2c[cc][:, j, :],
                                start=(amt == 0),
                                stop=(amt == NT - 1),
                                tile_position=(0, 32 * q),
                            )

                    def emit_av(amt=amt, aet=aet, p2c=p2c, av=av):
                        for c in range(2):
                            acs = slice(c * 512, (c + 1) * 512)
                            for j in range(2):
                                nc.tensor.matmul(
                                    av[j * 64 : (j + 1) * 64, acs],
                                    lhsT=v_t[amt][:, 2 * aet + j, :],
                                    rhs=p2c[c][:, j, :],
                                    start=(amt == 0),
                                    stop=(amt == NT - 1),
                                    tile_position=(0, 64 * j),
                                )

                    # at row starts the av banks are freed by the avr copy
                    # only ~1.5us into the group: emit AV after the fillers
                    # so the PE doesn't stall on the bank WAR
                    if ar == 0 and s2 < NSTEPS:
                        deferred_av = emit_av
                    else:
                        emit_av()
                    if ar == NSTEP - 2:
                        # avr eviction: two 512-col halves slotted between
                        # the exps of this and the next group (each half
                        # fits the exp-chain slack; a full copy would delay
                        # exp c1 enough to stall the next group's scores)
                        avr = avrp.tile([128, L], bf16, tag="avr", name="avr")
                        if s2 >= NSTEPS - 2:
                            nc.scalar.copy(out=avr[:, :], in_=av[:, :])
                        else:
                            # c0 half on scalar (fits the between-exps
                            # slack); c1 half on vector so the second
                            # group's exp chain is not delayed (that delay
                            # surfaced as a ~613ns scores stall at the
                            # light flush-only group two steps later)
                            avr_jobs.append(
                                lambda avr=avr, av=av: nc.scalar.copy(
                                    out=avr[:, 0:512], in_=av[:, 0:512]
                                )
                            )
                            avr_jobs.append(
                                lambda avr=avr, av=av: nc.vector.tensor_copy(
                                    out=avr[:, 512:1024], in_=av[:, 512:1024]
                                )
                            )
                        rz = smp.tile([128, 512], f32, tag="rz", name="rz")
                        nc.vector.reciprocal_approx_fast(out=rz[:, :], in_=zt[:, :])
                        rzb = smp.tile([128, 512], bf16, tag="rzb", name="rzb")
                        nc.vector.tensor_copy(out=rzb[:, :], in_=rz[:, :])
                        pending = (aet, avr, rzb)
                if s2 < NSTEPS:
                    et, r = divmod(s2, NSTEP)
                    mt = r // 2
                    if r == 8 and pending is not None:
                        flushing = pending
                        pending = None
                        rb_cur = rbp.tile([128, L], bf16, name="rb")
                        flush_half(flushing, 0, rb_cur)
                    if r == 10 and flushing is not None:
                        flush_half(flushing, 1, rb_cur)
                        flushing = None
                    # fillers: row 0 hosts V4..V7 c0-halves (heads 0-7,
                    # all rows 0-3 need only these); c1-halves spread over
                    # rows 0-3 (first needed by row 4)
                    if et == 0 and r <= 6:
                        v_proj_half(4 + r // 2, 0)
                    if et == 0 and r == 8:
                        v_proj_half(4, 1)
                    if 1 <= et <= 3 and r == 0:
                        v_proj_half(4 + et, 1)
                    if et >= 1:
                        # K c1 for this row: rows 1-3 burst it at r=2 (r=0
                        # holds their V half); rows 4-7 spread quarters over
                        # r=0/r=2 so no slot is left empty before scores
                        if et <= 3:
                            if r == 2:
                                kq_proj_half(wk_t, kt_t, et, 1)
                        elif r in (0, 2):
                            kq_proj_quarter(wk_t, kt_t, et, 1, r // 2)
                    if et + 1 < NT:
                        if et >= 1 and r in (4, 6):
                            kq_proj_quarter(wq_t, qt_t, et + 1, 0, (r - 4) // 2)
                        elif et == 0 and r == 10:
                            kq_proj_half(wq_t, qt_t, 1, 0)
                        elif r == 12:
                            kq_proj_half(wq_t, qt_t, et + 1, 1)
                        elif r == 14:
                            kq_proj_half(wk_t, kt_t, et + 1, 0)
                    if deferred_av is not None:
                        deferred_av()
                        deferred_av = None
                    # scores last (64x128 row-tiled mode)
                    pscs = []
                    for c in range(2):
                        psc = scp.tile([128, L], f32, tag="sc", name="sps")
                        pscs.append(psc)
                    for c in range(2):
                        cs = slice(c * 512, (c + 1) * 512)
                        for j in range(2):
                            jp = slice(j * 64, (j + 1) * 64)
                            nc.tensor.matmul(
                                pscs[c][:, j * 512 : (j + 1) * 512],
                                lhsT=kt_t[et][jp, mt * 128 : (mt + 1) * 128],
                                rhs=qt_t[et][jp, cs],
                                start=True,
                                stop=True,
                            )
                    for c in range(2):
                        cs = slice(c * 512, (c + 1) * 512)
                        p2 = pbp.tile([128, 2, 512], bf16, name="p2")
                        nc.scalar.activation(
                            p2[:, :, :],
                            pscs[c][:, :].rearrange("p (j l) -> p j l", j=2),
                            Exp, scale=SCALE, bias=negC[:, :],
                        )
                        if c == 0 and avr_jobs:
                            avr_jobs.pop(0)()
                        nc.vector.tensor_mul(
                            p2[:, :, :],
                            p2[:, :, :],
                            mk_t[mt][:, cs].rearrange("p (o l) -> p o l", o=1).to_broadcast([128, 2, 512]),
                        )
                        pref[(et, r + c)] = p2


            # ---- output projection, overlapped with the final flush:
            # lt=0,1 accumulate p=0..6 first (ot_t[7] not ready yet), the
            # flush (which waits on the reciprocal) runs behind them, then
            # the p=7 matmuls complete those tiles.
            early = []
            for lt in range(2):
                ps = scp.tile([128, L], f32, tag="sc", name="yps")
                for fc in range(2):
                    cs = slice(fc * 512, (fc + 1) * 512)
                    for p in range(NT - 1):
                        for h in range(2):
                            nc.tensor.matmul(
                                ps[64 * h : 64 * (h + 1), cs],
                                lhsT=ot_t[p][
                                    :, lt * 128 + 64 * h : lt * 128 + 64 * (h + 1)
                                ],
                                rhs=wo_t[p][:, cs],
                                start=(p == 0),
                                stop=False,
                                tile_position=(0, 64 * h),
                            )
                early.append(ps)

            if pending is not None:
                rb_f = rbp.tile([128, L], bf16, name="rbf")
                flush_half(pending, 0, rb_f)
                flush_half(pending, 1, rb_f)
                pending = None

            for lt in range(NT):
                if lt < 2:
                    ps = early[lt]
                else:
                    ps = scp.tile([128, L], f32, tag="sc", name="yps")
                for fc in range(2):
                    cs = slice(fc * 512, (fc + 1) * 512)
                    ps_range = range(NT - 1, NT) if lt < 2 else range(NT)
                    for p in ps_range:
                        for h in range(2):
                            nc.tensor.matmul(
                                ps[64 * h : 64 * (h + 1), cs],
                                lhsT=ot_t[p][
                                    :, lt * 128 + 64 * h : lt * 128 + 64 * (h + 1)
                                ],
                                rhs=wo_t[p][:, cs],
                                start=(p == 0 and lt >= 2),
                                stop=(p == NT - 1),
                                tile_position=(0, 64 * h),
                            )
                y = ybp.tile([128, L], f16, name="y")
                # alternate eviction engines so the tail drains in parallel
                if lt % 2 == 0:
                    nc.scalar.copy(out=y[:, :], in_=ps[:, :])
                else:
                    nc.vector.tensor_copy(out=y[:, :], in_=ps[:, :])
                nc.sync.dma_start(out=out[lt * 128 : (lt + 1) * 128, :], in_=y[:, :])

    nc.finalize()
    return nc


def _get_nc():
    global _NC_CACHE
    if _NC_CACHE is None:
        _NC_CACHE = _build()
    return _NC_CACHE


def _make_in_maps(x, mask, Wk, Wv, Wq, Wo):
    f16 = np.float16
    bf16 = ml_dtypes.bfloat16
    wqT = np.ascontiguousarray(Wq.T).astype(f16)
    wkT = np.ascontiguousarray(Wk.T).astype(f16)
    wvT = np.ascontiguousarray(Wv.T).astype(bf16)
    woT = np.ascontiguousarray(Wo.T).astype(f16)
    maskT = np.ascontiguousarray(mask[0].T).astype(bf16)
    in_maps = []
    for b in range(N_CORES):
        in_maps.append(
            {
                "xT": np.ascontiguousarray(x[b].T).astype(f16),
                "wqT": wqT,
                "wkT": wkT,
                "wvT": wvT,
                "woT": woT,
                "maskT": maskT,
            }
        )
    return in_maps


def _run(x, mask, Wk, Wv, Wq, Wo, trace=False):
    from concourse.bass_utils import run_bass_kernel_spmd

    nc = _get_nc()
    in_maps = _make_in_maps(x, mask, Wk, Wv, Wq, Wo)
    res = run_bass_kernel_spmd(nc, in_maps, list(range(N_CORES)), trace=trace)
    y = np.stack([res.results[b]["out"] for b in range(N_CORES)], axis=0)
    return y.astype(np.float32), res


def kernel(x, mask, Wk, Wv, Wq, Wo):
    y, _ = _run(x, mask, Wk, Wv, Wq, Wo, trace=False)
    return y



# revision 20
# speedup vs baseline: 1.0719x; 1.0014x over previous
"""Trainium2 Bass kernel for batched multi-head attention.

HW exec time: 256.1us full-clock (v5 baseline 264us full-clock /
313us throttled). Verified rel err 0.0037. v6: col-tiled (128x64)
projection pairs (kills the 77ns/MM full-mode weight-swap gap), 3
tiling-mode switches/group with scores emitted last, sel-matrix flush
sharing the AV mode, DMA ordering by first use, HAM warmup MMs.
v8: kc1/q0 fillers as 8-MM quarters over slots r0-r6 with flushes at
r8/r10 (kills the r2-pocket scores stall), avr eviction halves split
scalar (c0, between exps) / vector (c1, so the second group's exp
chain is not delayed - that delay resurfaced as a 613ns scores stall
at the light flush-only group).

Distribution: pure data parallel - batch b -> NeuronCore b.

v5 vs v4 (275.5us): trailing AV/Z emitted first in each 2-step
group (ready work leads the PE FIFO; sem-waiting scores no longer
block it), TRAIL=6.

v4 vs v3 (278us):
  - 2-step grouping with same-shape matmul clustering: scores for both
    c-chunks of a key-tile emitted as one 4-MM same-shape burst, AV
    likewise; LDWEIGHTS pipelines within a same-shape burst (~224ns/slot
    vs ~302ns/slot mixed).
  - TRAIL=4 so the trailing AV/Z work aligns to (mt) pairs.
  - prefix shortened: V0,V1,K0,V2,V3,Q0 up front (first score at
    ~34us instead of ~48us); V4..V7 projected inside row 0 as 8-MM
    half-chunks in the 1-bank work pool; K1/Q1 fillers at r=8..14.

PSUM: scores 2x[128,1024] (4 banks), AV [128,1024] (2), Z [97,512] (1),
work [128,512] (1) = 8 banks.
"""

import sys

if "/opt/trn_rl_repo" not in sys.path:
    sys.path.insert(0, "/opt/trn_rl_repo")

import numpy as np
import ml_dtypes

B = 8
L = 1024
D = 1024
H = 16
DH = 64
NT = 8
SCALE = 0.125
EXP_SHIFT = -32.0
TRAIL = 6  # AV/Z trail the scores by this many steps (even: aligns mt pairs)
N_CORES = 8
NSTEP = 2 * NT

_NC_CACHE = None


def _build():
    import concourse.bass as bass  # noqa: F401
    from concourse import bacc, mybir
    import concourse.tile as tile

    f32 = mybir.dt.float32
    f16 = mybir.dt.float16
    bf16 = mybir.dt.bfloat16
    Exp = mybir.ActivationFunctionType.Exp

    nc = bacc.Bacc(None, target_bir_lowering=False)

    xT = nc.declare_dram_parameter("xT", [D, L], f16, isOutput=False)
    wqT = nc.declare_dram_parameter("wqT", [D, D], f16, isOutput=False)
    wkT = nc.declare_dram_parameter("wkT", [D, D], f16, isOutput=False)
    wvT = nc.declare_dram_parameter("wvT", [D, D], bf16, isOutput=False)
    woT = nc.declare_dram_parameter("woT", [D, D], f16, isOutput=False)
    mskT = nc.declare_dram_parameter("maskT", [L, L], bf16, isOutput=False)
    out = nc.declare_dram_parameter("out", [L, D], f16, isOutput=True)

    with tile.TileContext(nc) as tc:
        with (
            tc.tile_pool(name="big", bufs=1) as big,
            tc.tile_pool(name="pb", bufs=TRAIL + 3) as pbp,
            tc.tile_pool(name="sm", bufs=2) as smp,
            tc.tile_pool(name="rb", bufs=2) as rbp,
            tc.tile_pool(name="avr", bufs=2) as avrp,
            tc.tile_pool(name="yb", bufs=2) as ybp,
            tc.tile_pool(name="sc", bufs=2, space="PSUM") as scp,
            tc.tile_pool(name="pav", bufs=1, space="PSUM") as pavp,
            tc.tile_pool(name="pz", bufs=1, space="PSUM") as pzp,
            tc.tile_pool(name="wk", bufs=1, space="PSUM") as wkp,
        ):
            x_t = [big.tile([128, L], f16, tag=f"x{i}", name=f"x{i}") for i in range(NT)]
            wq_t = [big.tile([128, D], f16, tag=f"wq{i}", name=f"wq{i}") for i in range(NT)]
            wk_t = [big.tile([128, D], f16, tag=f"wk{i}", name=f"wk{i}") for i in range(NT)]
            wv_t = [big.tile([128, D], bf16, tag=f"wv{i}", name=f"wv{i}") for i in range(NT)]
            wo_t = [big.tile([128, D], f16, tag=f"wo{i}", name=f"wo{i}") for i in range(NT)]
            mk_t = [big.tile([128, L], bf16, tag=f"mk{i}", name=f"mk{i}") for i in range(NT)]
            qt_t = [big.tile([128, L], f16, tag=f"qt{i}", name=f"qt{i}") for i in range(NT)]
            kt_t = [big.tile([128, L], f16, tag=f"kt{i}", name=f"kt{i}") for i in range(NT)]
            v_t = [big.tile([128, H, DH], bf16, tag=f"v{i}", name=f"v{i}") for i in range(NT)]
            ot_t = [big.tile([128, L], f16, tag=f"ot{i}", name=f"ot{i}") for i in range(NT)]
            ones32 = big.tile([128, 32], bf16, tag="ones32")
            sel = big.tile([128, 4, 64], bf16, tag="sel")
            negC = big.tile([128, 1], f32, tag="negC")

            # input DMAs, ordered by first use and striped so early tiles
            # land first: x+wv quarters (V0 starts ~3us in), then the col-0
            # chunks of wk/wq (all K0/Q0 needs), mask tiles, then the rest.
            for i in range(NT):
                sl = slice(i * 128, (i + 1) * 128)
                nc.sync.dma_start(out=wv_t[i][:, :], in_=wvT[sl, :])
                nc.sync.dma_start(out=x_t[i][:, :], in_=xT[sl, :])
            for i in range(NT):
                sl = slice(i * 128, (i + 1) * 128)
                nc.sync.dma_start(out=wk_t[i][:, 0:128], in_=wkT[sl, 0:128])
                nc.sync.dma_start(out=wq_t[i][:, 0:128], in_=wqT[sl, 0:128])
            for i in range(NT):
                sl = slice(i * 128, (i + 1) * 128)
                nc.sync.dma_start(out=mk_t[i][:, :], in_=mskT[sl, :])
            for i in range(NT):
                sl = slice(i * 128, (i + 1) * 128)
                nc.sync.dma_start(out=wk_t[i][:, 128:1024], in_=wkT[sl, 128:1024])
                nc.sync.dma_start(out=wq_t[i][:, 128:1024], in_=wqT[sl, 128:1024])
            for i in range(NT):
                sl = slice(i * 128, (i + 1) * 128)
                nc.sync.dma_start(out=wo_t[i][:, :], in_=woT[sl, :])

            nc.vector.memset(ones32[:, :], 1.0)
            nc.vector.memset(sel[:, :, :], 0.0)
            for q in range(4):
                nc.vector.memset(sel[32 * q : 32 * q + 1, q, :], 1.0)
            nc.vector.memset(negC[:, :], EXP_SHIFT)

            def proj_full(lhs_fn, rhs_fn, evict):
                # col-tiled (128x64 mode): two concurrent 64-col tiles per
                # 512-col step; tiled rounds chain with ~0 gap vs 77ns/MM in
                # full 128x128 mode.
                ps = scp.tile([128, L], f32, tag="sc", name="projps")
                for c in range(2):
                    cs = slice(c * 512, (c + 1) * 512)
                    for dt in range(NT):
                        for h in range(2):
                            nc.tensor.matmul(
                                ps[64 * h : 64 * (h + 1), cs],
                                lhsT=lhs_fn(dt, h),
                                rhs=rhs_fn(dt, cs),
                                start=(dt == 0),
                                stop=(dt == NT - 1),
                                tile_position=(0, 64 * h),
                            )
                evict(ps)

            def v_proj(mt):
                proj_full(
                    lambda dt, h: x_t[dt][:, mt * 128 + 64 * h : mt * 128 + 64 * (h + 1)],
                    lambda dt, cs: wv_t[dt][:, cs],
                    lambda ps: nc.vector.tensor_copy(
                        out=v_t[mt][:, :, :],
                        in_=ps[:, :].rearrange("p (h e) -> p h e", h=H),
                    ),
                )

            def kq_proj(w_tiles, dst, et):
                proj_full(
                    lambda dt, h: w_tiles[dt][:, et * 128 + 64 * h : et * 128 + 64 * (h + 1)],
                    lambda dt, cs: x_t[dt][:, cs],
                    lambda ps: nc.vector.tensor_copy(out=dst[et][:, :], in_=ps[:, :]),
                )

            hp_open = {}

            def kq_proj_quarter(w_tiles, dst, et, c, half):
                # 8-MM quarter of a kq projection half: spreads filler work
                # more uniformly over group slots so light groups don't
                # expose the scalar exp-chain lag as pre-scores PE stalls
                key = (id(w_tiles), et, c)
                cs = slice(c * 512, (c + 1) * 512)
                if half == 0:
                    hp_open[key] = wkp.tile([128, 512], f32, tag="wk", name="hp")
                ps = hp_open[key]
                for dt in range(4 * half, 4 * half + 4):
                    for h in range(2):
                        nc.tensor.matmul(
                            ps[64 * h : 64 * (h + 1), :],
                            lhsT=w_tiles[dt][
                                :, et * 128 + 64 * h : et * 128 + 64 * (h + 1)
                            ],
                            rhs=x_t[dt][:, cs],
                            start=(dt == 0),
                            stop=(dt == NT - 1),
                            tile_position=(0, 64 * h),
                        )
                if half == 1:
                    nc.vector.tensor_copy(out=dst[et][:, cs], in_=ps[:, :])
                    del hp_open[key]

            def kq_proj_half(w_tiles, dst, et, c):
                kq_proj_quarter(w_tiles, dst, et, c, 0)
                kq_proj_quarter(w_tiles, dst, et, c, 1)

            def v_proj_half(mt, c):
                # half of V proj: output heads c*8..c*8+8 of v_t[mt]
                ps = wkp.tile([128, 512], f32, tag="wk", name="vh")
                cs = slice(c * 512, (c + 1) * 512)
                for dt in range(NT):
                    for h in range(2):
                        nc.tensor.matmul(
                            ps[64 * h : 64 * (h + 1), :],
                            lhsT=x_t[dt][
                                :, mt * 128 + 64 * h : mt * 128 + 64 * (h + 1)
                            ],
                            rhs=wv_t[dt][:, cs],
                            start=(dt == 0),
                            stop=(dt == NT - 1),
                            tile_position=(0, 64 * h),
                        )
                nc.vector.tensor_copy(
                    out=v_t[mt][:, c * 8 : (c + 1) * 8, :],
                    in_=ps[:, :].rearrange("p (h e) -> p h e", h=8),
                )

            # ---- prefix: V0,V1,K0,V2,V3,Q0 (V4..V7 run inside row 0)
            v_proj(0)
            v_proj(1)
            kq_proj(wk_t, kt_t, 0)
            v_proj(2)
            v_proj(3)
            kq_proj(wq_t, qt_t, 0)

            # ---- head: 64 groups of 2 steps: (et, mt) with c=0,1
            def flush_half(p, c, rb):
                # broadcast reciprocal rows via sel matrices in the same
                # (128,64) col-tiled mode as AV/fillers (no PE mode switch)
                pet, pavr, przb = p
                cs = slice(c * 512, (c + 1) * 512)
                psr = wkp.tile([128, 512], f32, tag="wk", name="psr")
                for j in range(2):
                    q = 2 * j + c
                    nc.tensor.matmul(
                        psr[64 * j : 64 * j + 64, :],
                        lhsT=sel[:, q, :],
                        rhs=przb[:, :],
                        start=True,
                        stop=True,
                        tile_position=(0, 64 * j),
                    )
                nc.vector.tensor_copy(out=rb[:, cs], in_=psr[:, :])
                if c == 1:
                    nc.vector.tensor_mul(ot_t[pet][:, :], pavr[:, :], rb[:, :])

            pending = None
            flushing = None
            rb_cur = None
            av = None
            zt = None
            avr_jobs = []
            deferred_av = None
            pref = {}
            NSTEPS = NT * NT * 2

            # ---- HAM warmup: ~4us of dummy matmuls on resident data while
            # the input DMAs stream, so the PE clock-gate opens (1.2->2.4GHz)
            # before the first real projection.
            wps = wkp.tile([128, 512], f32, tag="wk", name="warm")
            for wi in range(20):
                nc.tensor.matmul(
                    wps[0:64, 0:256],
                    lhsT=sel[:, 0, :],
                    rhs=sel[:, :, :].rearrange("p a b -> p (a b)"),
                    start=(wi == 0),
                    stop=(wi == 19),
                    tile_position=(0, 0),
                )
            # per-group emission: [Z (128x32)] [AV+flush+fillers (128x64)]
            # [scores (64x128)] — 3 mode switches/group, and scores go last
            # so the PE has a full group of work queued while waiting for
            # exp(g-1) to free the scores PSUM pool.
            for s2 in range(0, NSTEPS + TRAIL, 2):
                as2 = s2 - TRAIL
                if as2 >= 0:
                    aet, ar = divmod(as2, NSTEP)
                    amt = ar // 2
                    if ar == 0:
                        av = pavp.tile([128, L], f32, tag="av", name="av")
                        zt = pzp.tile([128, 512], f32, tag="z", name="z")
                    p2c = [pref.pop((aet, ar)), pref.pop((aet, ar + 1))]
                    for j in range(2):
                        for cc in range(2):
                            q = 2 * j + cc
                            nc.tensor.matmul(
                                zt[32 * q : 32 * q + 32, :],
                                lhsT=ones32[:, :],
                                rhs=p2c[cc][:, j, :],
                                start=(amt == 0),
                                stop=(amt == NT - 1),
                                tile_position=(0, 32 * q),
                            )

                    def emit_av(amt=amt, aet=aet, p2c=p2c, av=av):
                        for c in range(2):
                            acs = slice(c * 512, (c + 1) * 512)
                            for j in range(2):
                                nc.tensor.matmul(
                                    av[j * 64 : (j + 1) * 64, acs],
                                    lhsT=v_t[amt][:, 2 * aet + j, :],
                                    rhs=p2c[c][:, j, :],
                                    start=(amt == 0),
                                    stop=(amt == NT - 1),
                                    tile_position=(0, 64 * j),
                                )

                    # at row starts the av banks are freed by the avr copy
                    # only ~1.5us into the group: emit AV after the fillers
                    # so the PE doesn't stall on the bank WAR
                    if ar == 0 and s2 < NSTEPS:
                        deferred_av = emit_av
                    else:
                        emit_av()
                    if ar == NSTEP - 2:
                        # avr eviction: two 512-col halves slotted between
                        # the exps of this and the next group (each half
                        # fits the exp-chain slack; a full copy would delay
                        # exp c1 enough to stall the next group's scores)
                        avr = avrp.tile([128, L], bf16, tag="avr", name="avr")
                        if s2 >= NSTEPS - 2:
                            nc.scalar.copy(out=avr[:, :], in_=av[:, :])
                        else:
                            # c0 half on scalar (fits the between-exps
                            # slack); c1 half on vector so the second
                            # group's exp chain is not delayed (that delay
                            # surfaced as a ~613ns scores stall at the
                            # light flush-only group two steps later)
                            avr_jobs.append(
                                lambda avr=avr, av=av: nc.scalar.copy(
                                    out=avr[:, 0:512], in_=av[:, 0:512]
                                )
                            )
                            avr_jobs.append(
                                lambda avr=avr, av=av: nc.vector.tensor_copy(
                                    out=avr[:, 512:1024], in_=av[:, 512:1024]
                                )
                            )
                        rz = smp.tile([128, 512], f32, tag="rz", name="rz")
                        nc.vector.reciprocal_approx_fast(out=rz[:, :], in_=zt[:, :])
                        rzb = smp.tile([128, 512], bf16, tag="rzb", name="rzb")
                        nc.vector.tensor_copy(out=rzb[:, :], in_=rz[:, :])
                        pending = (aet, avr, rzb)
                if s2 < NSTEPS:
                    et, r = divmod(s2, NSTEP)
                    mt = r // 2
                    if r == 8 and pending is not None:
                        flushing = pending
                        pending = None
                        rb_cur = rbp.tile([128, L], bf16, name="rb")
                        flush_half(flushing, 0, rb_cur)
                    if r == 10 and flushing is not None:
                        flush_half(flushing, 1, rb_cur)
                        flushing = None
                    # fillers: row 0 hosts V4..V7 c0-halves (heads 0-7,
                    # all rows 0-3 need only these); c1-halves spread over
                    # rows 0-3 (first needed by row 4)
                    if et == 0 and r <= 6:
                        v_proj_half(4 + r // 2, 0)
                    if et == 0 and r == 8:
                        v_proj_half(4, 1)
                    if 1 <= et <= 3 and r == 0:
                        v_proj_half(4 + et, 1)
                    if et >= 1:
                        # K c1 for this row: rows 1-3 burst it at r=2 (r=0
                        # holds their V half); rows 4-7 spread quarters over
                        # r=0/r=2 so no slot is left empty before scores
                        if et <= 3:
                            if r == 2:
                                kq_proj_half(wk_t, kt_t, et, 1)
                        elif r in (0, 2):
                            kq_proj_quarter(wk_t, kt_t, et, 1, r // 2)
                    if et + 1 < NT:
                        if et >= 1 and r in (4, 6):
                            kq_proj_quarter(wq_t, qt_t, et + 1, 0, (r - 4) // 2)
                        elif et == 0 and r == 10:
                            kq_proj_half(wq_t, qt_t, 1, 0)
                        elif r == 12:
                            kq_proj_half(wq_t, qt_t, et + 1, 1)
                        elif r == 14:
                            kq_proj_half(wk_t, kt_t, et + 1, 0)
                    if deferred_av is not None:
                        deferred_av()
                        deferred_av = None
                    # scores last (64x128 row-tiled mode)
                    pscs = []
                    for c in range(2):
                        psc = scp.tile([128, L], f32, tag="sc", name="sps")
                        pscs.append(psc)
                    for c in range(2):
                        cs = slice(c * 512, (c + 1) * 512)
                        for j in range(2):
                            jp = slice(j * 64, (j + 1) * 64)
                            nc.tensor.matmul(
                                pscs[c][:, j * 512 : (j + 1) * 512],
                                lhsT=kt_t[et][jp, mt * 128 : (mt + 1) * 128],
                                rhs=qt_t[et][jp, cs],
                                start=True,
                                stop=True,
                            )
                    for c in range(2):
                        cs = slice(c * 512, (c + 1) * 512)
                        p2 = pbp.tile([128, 2, 512], bf16, name="p2")
                        nc.scalar.activation(
                            p2[:, :, :],
                            pscs[c][:, :].rearrange("p (j l) -> p j l", j=2),
                            Exp, scale=SCALE, bias=negC[:, :],
                        )
                        if c == 0 and avr_jobs:
                            avr_jobs.pop(0)()
                        nc.vector.tensor_mul(
                            p2[:, :, :],
                            p2[:, :, :],
                            mk_t[mt][:, cs].rearrange("p (o l) -> p o l", o=1).to_broadcast([128, 2, 512]),
                        )
                        pref[(et, r + c)] = p2


            # ---- output projection, overlapped with the final flush:
            # lt=0,1 accumulate p=0..6 first (ot_t[7] not ready yet), the
            # flush (which waits on the reciprocal) runs behind them, then
            # the p=7 matmuls complete those tiles.
            early = []
            for lt in range(2):
                ps = scp.tile([128, L], f32, tag="sc", name="yps")
                for fc in range(2):
                    cs = slice(fc * 512, (fc + 1) * 512)
                    for p in range(NT - 1):
                        for h in range(2):
                            nc.tensor.matmul(
                                ps[64 * h : 64 * (h + 1), cs],
                                lhsT=ot_t[p][
                                    :, lt * 128 + 64 * h : lt * 128 + 64 * (h + 1)
                                ],
                                rhs=wo_t[p][:, cs],
                                start=(p == 0),
                                stop=False,
                                tile_position=(0, 64 * h),
                            )
                early.append(ps)

            if pending is not None:
                rb_f = rbp.tile([128, L], bf16, name="rbf")
                flush_half(pending, 0, rb_f)
                flush_half(pending, 1, rb_f)
                pending = None

            for lt in range(NT):
                if lt < 2:
                    ps = early[lt]
                else:
                    ps = scp.tile([128, L], f32, tag="sc", name="yps")
                for fc in range(2):
                    cs = slice(fc * 512, (fc + 1) * 512)
                    ps_range = range(NT - 1, NT) if lt < 2 else range(NT)
                    for p in ps_range:
                        for h in range(2):
                            nc.tensor.matmul(
                                ps[64 * h : 64 * (h + 1), cs],
                                lhsT=ot_t[p][
                                    :, lt * 128 + 64 * h : lt * 128 + 64 * (h + 1)
                                ],
                                rhs=wo_t[p][:, cs],
                                start=(p == 0 and lt >= 2),
                                stop=(p == NT - 1),
                                tile_position=(0, 64 * h),
                            )
                y = ybp.tile([128, L], f16, name="y")
                # alternate eviction engines so the tail drains in parallel
                if lt % 2 == 0:
                    nc.scalar.copy(out=y[:, :], in_=ps[:, :])
                else:
                    nc.vector.tensor_copy(out=y[:, :], in_=ps[:, :])
                nc.sync.dma_start(out=out[lt * 128 : (lt + 1) * 128, :], in_=y[:, :])

    nc.finalize()
    return nc


def _get_nc():
    global _NC_CACHE
    if _NC_CACHE is None:
        _NC_CACHE = _build()
    return _NC_CACHE


def _make_in_maps(x, mask, Wk, Wv, Wq, Wo):
    f16 = np.float16
    bf16 = ml_dtypes.bfloat16
    wqT = np.ascontiguousarray(Wq.T).astype(f16)
    wkT = np.ascontiguousarray(Wk.T).astype(f16)
    wvT = np.ascontiguousarray(Wv.T).astype(bf16)
    woT = np.ascontiguousarray(Wo.T).astype(f16)
    maskT = np.ascontiguousarray(mask[0].T).astype(bf16)
    in_maps = []
    for b in range(N_CORES):
        in_maps.append(
            {
                "xT": np.ascontiguousarray(x[b].T).astype(f16),
                "wqT": wqT,
                "wkT": wkT,
                "wvT": wvT,
                "woT": woT,
                "maskT": maskT,
            }
        )
    return in_maps


def _run(x, mask, Wk, Wv, Wq, Wo, trace=False):
    from concourse.bass_utils import run_bass_kernel_spmd

    nc = _get_nc()
    in_maps = _make_in_maps(x, mask, Wk, Wv, Wq, Wo)
    res = run_bass_kernel_spmd(nc, in_maps, list(range(N_CORES)), trace=trace)
    y = np.stack([res.results[b]["out"] for b in range(N_CORES)], axis=0)
    return y.astype(np.float32), res


def kernel(x, mask, Wk, Wv, Wq, Wo):
    y, _ = _run(x, mask, Wk, Wv, Wq, Wo, trace=False)
    return y

